# Initial kernel scaffold
#
"""APNB (asymmetric pyramid non-local block) Trainium2 kernel.

Full inputs in, full output out. Sharding: 8 cores = (batch b=core//2,
H-half=core%2). Each core handles feats[b, :, 64*half:64*half+64, :]
(= [512, 8192] pixels), computes q/v projections, pools q/v into the
110 PPM positions (partial sums, AllReduced across the 2 cores of the
same batch), runs the 110-position attention over its 8192 query
pixels, and the fused output projection + bottleneck conv.
"""

import sys

for p in ("/opt/trn_rl_repo",):
    if p not in sys.path:
        sys.path.insert(0, p)

import numpy as np
import ml_dtypes

BF16 = ml_dtypes.bfloat16

# ---- problem constants (hardcoded per spec) ----
B, C, H, W = 4, 512, 128, 128
CK, CO = 256, 512
POOL_SCALES = (1, 3, 6, 8)
S_TOT = sum(s * s for s in POOL_SCALES)  # 110
N_CORES = 8
H_SH = H // 2          # 64 rows per core
NPIX = H_SH * W        # 8192 pixels per core
PIX_T = 512            # gemm pixel tile
N_T = NPIX // PIX_T    # 16
N_ROWS = H_SH          # 64 (one transpose block per image row)


def _pool_bounds(n, s):
    i = np.arange(s)
    return (i * n) // s, -((-(i + 1) * n) // s)


def _build_pool_mat(h0):
    """P[w, r, bin] = 1 if global pixel (h0+r, w) is in bin. bf16."""
    P = np.zeros((W, H_SH, S_TOT), dtype=np.float32)
    inv_area = np.zeros((S_TOT,), dtype=np.float32)
    off = 0
    for s in POOL_SCALES:
        hs, he = _pool_bounds(H, s)
        ws, we = _pool_bounds(W, s)
        for i in range(s):
            for j in range(s):
                b = off + i * s + j
                inv_area[b] = 1.0 / float((he[i] - hs[i]) * (we[j] - ws[j]))
                r0 = max(hs[i] - h0, 0)
                r1 = min(he[i] - h0, H_SH)
                if r1 > r0:
                    P[ws[j]:we[j], r0:r1, b] = 1.0
        off += s * s
    return P.astype(BF16), inv_area


KC_CONST = C // 128
KK_CONST = CK // 128


def _build_kernel(sim_mode=False):
    import concourse.bass as bass
    import concourse.bacc as bacc
    import concourse.mybir as mybir
    from concourse import tile, masks

    dt = mybir.dt
    f32, bf16 = dt.float32, dt.bfloat16
    AF = mybir.ActivationFunctionType
    ALU = mybir.AluOpType

    nc = bacc.Bacc("TRN2", target_bir_lowering=False, debug=False)

    # ---- DRAM I/O ----
    feats_d = nc.dram_tensor("feats", [C, NPIX], bf16, kind="ExternalInput")
    wqv_d = nc.dram_tensor("w_qv", [128, 2 * KC_CONST * CK], bf16, kind="ExternalInput")
    wob_d = nc.dram_tensor("w_ob", [128, (KC_CONST + KK_CONST) * CO], bf16, kind="ExternalInput")
    bias_d = nc.dram_tensor("biases", [128, 10], f32, kind="ExternalInput")
    pmat_d = nc.dram_tensor("pmat", [W, N_ROWS, S_TOT], bf16, kind="ExternalInput")
    out_d = nc.dram_tensor("out", [CO, NPIX], bf16, kind="ExternalOutput")

    KC = C // 128   # 4 k-chunks over input channels
    KK = CK // 128  # 2 chunks over attention channels
    MO = CO // 128  # 4 m-chunks over output channels
    assert (KC, KK) == (KC_CONST, KK_CONST)

    with tile.TileContext(nc) as tc:
        with (
            tc.tile_pool(name="const", bufs=1) as const_pool,
            tc.tile_pool(name="resident", bufs=1) as res_pool,
            tc.tile_pool(name="work", bufs=6) as work_pool,
            tc.tile_pool(name="attn", bufs=6) as attn_pool,
            tc.tile_pool(name="outb", bufs=2) as out_pool,
            tc.tile_pool(name="psmall", bufs=4, space="PSUM") as psmall,
            tc.tile_pool(name="pbig", bufs=2, space="PSUM") as pbig,
            tc.tile_pool(name="pacc", bufs=1, space="PSUM") as pacc,
            tc.tile_pool(name="pctx", bufs=1, space="PSUM") as pctx,
            tc.tile_pool(name="dram", bufs=1, space="DRAM") as dram_pool,
        ):
            # ---- constants / weights ----
            ident = const_pool.tile([128, 128], bf16, tag="ident", name="ident")
            masks.make_identity(nc, ident[:])

            wqv_sb = const_pool.tile([128, 2 * KC * CK], bf16, tag="wqv", name="wqv")
            nc.sync.dma_start(wqv_sb[:], wqv_d[:])
            bias_sb = const_pool.tile([128, 10], f32, tag="bias", name="bias")
            nc.sync.dma_start(bias_sb[:], bias_d[:])
            wob_sb = const_pool.tile([128, (KC + KK) * CO], bf16, tag="wob", name="wob")
            # views: wkq chunk k -> [:, k*CK : k*CK+CK]; wv -> offset KC*CK
            wkq_sb = [wqv_sb[:, k * CK:(k + 1) * CK] for k in range(KC)]
            wv_sb = [wqv_sb[:, (KC + k) * CK:(KC + k + 1) * CK] for k in range(KC)]
            wbf_sb = [wob_sb[:, k * CO:(k + 1) * CO] for k in range(KC)]
            wco_sb = [wob_sb[:, (KC + k) * CO:(KC + k + 1) * CO] for k in range(KK)]
            bkq_sb = [bias_sb[:, k:k + 1] for k in range(KK)]
            bv_sb = [bias_sb[:, 2 + k:3 + k] for k in range(KK)]
            bco_sb = [bias_sb[:, 4 + m:5 + m] for m in range(MO)]
            iak_sb = bias_sb[:S_TOT, 8:9]
            iav_sb = bias_sb[:S_TOT, 9:10]
            pmat_sb = const_pool.tile([W, N_ROWS * S_TOT], bf16, tag="pmat", name="pmat")
            pmat_ap = pmat_sb[:].rearrange("w (r s) -> w r s", s=S_TOT)

            # ---- resident tensors ----
            feats_bf = [res_pool.tile([128, NPIX], bf16, tag=f"fbf{k}", name=f"fbf{k}") for k in range(KC)]
            q_bf = [res_pool.tile([128, NPIX], bf16, tag=f"qbf{k}", name=f"qbf{k}") for k in range(KK)]
            v_bf = [res_pool.tile([128, NPIX], bf16, tag=f"vbf{k}", name=f"vbf{k}") for k in range(KK)]

            # ---- phase A: load feats (bf16, host-cast), q/v gemms ----
            for t in range(N_T):
                px = bass.ts(t, PIX_T)
                if t % 2 == 0:
                    px2 = bass.ts(t // 2, 2 * PIX_T)
                    for k in range(KC):
                        nc.sync.dma_start(
                            feats_bf[k][:, px2], feats_d[k * 128:(k + 1) * 128, px2]
                        )
                for m in range(KK):
                    qp = pbig.tile([128, PIX_T], f32, tag="pbig", name="pbig")
                    for k in range(KC):
                        nc.tensor.matmul(
                            qp[:], wkq_sb[k][:, m * 128:(m + 1) * 128],
                            feats_bf[k][:, px], start=(k == 0), stop=(k == KC - 1),
                        )
                    nc.vector.tensor_scalar(
                        q_bf[m][:, px], qp[:], bkq_sb[m], 0.0, ALU.add, ALU.max
                    )
                for m in range(KK):
                    vp = pbig.tile([128, PIX_T], f32, tag="pbig", name="pbig")
                    for k in range(KC):
                        nc.tensor.matmul(
                            vp[:], wv_sb[k][:, m * 128:(m + 1) * 128],
                            feats_bf[k][:, px], start=(k == 0), stop=(k == KC - 1),
                        )
                    nc.vector.tensor_scalar(
                        v_bf[m][:, px], vp[:], bv_sb[m], None, ALU.add
                    )

            # deferred const loads (needed from phase B/D on)
            nc.sync.dma_start(
                pmat_sb[:], pmat_d.rearrange("w r s -> w (r s)")[:, :]
            )
            nc.sync.dma_start(wob_sb[:], wob_d[:])

            # ---- phase B: pool q and v -> partial bin sums [110, 512] ----
            pooled_ps = pacc.tile([S_TOT, 2 * CK], f32, tag="ps2", name="pooled_ps")
            for r in range(N_ROWS):
                rpx = bass.ts(r, 128)
                rowT = psmall.tile([128, 2 * CK], bf16, tag="ps1", name="rowT")
                for m in range(KK):
                    nc.tensor.transpose(
                        rowT[:, m * 128:(m + 1) * 128], q_bf[m][:, rpx], ident[:]
                    )
                    nc.tensor.transpose(
                        rowT[:, CK + m * 128:CK + (m + 1) * 128],
                        v_bf[m][:, rpx], ident[:],
                    )
                qvT = work_pool.tile([128, 2 * CK], bf16, tag="qvT", name="qvT")
                nc.scalar.copy(qvT[:, :CK], rowT[:, :CK])
                nc.vector.tensor_copy(qvT[:, CK:], rowT[:, CK:])
                nc.tensor.matmul(
                    pooled_ps[:], pmat_ap[:, r, :], qvT[:],
                    start=(r == 0), stop=(r == N_ROWS - 1),
                )


            # ---- phase C: AllReduce partial sums across the batch pair ----
            pooled_sb = work_pool.tile([S_TOT, 2 * CK], f32, tag="pooled", name="pooled")
            nc.vector.tensor_copy(pooled_sb[:], pooled_ps[:])
            cc_in = dram_pool.tile([S_TOT, 2 * CK], f32, tag="cc_in", name="cc_in")
            cc_out = dram_pool.tile([S_TOT, 2 * CK], f32, tag="cc_out", name="cc_out")
            nc.sync.dma_start(cc_in[:], pooled_sb[:])
            if sim_mode:
                nc.sync.dma_start(cc_out[:], cc_in[:])
            else:
                nc.gpsimd.collective_compute(
                    "AllReduce",
                    ALU.add,
                    replica_groups=[[0, 1], [2, 3], [4, 5], [6, 7]],
                    ins=[cc_in.opt()],
                    outs=[cc_out.opt()],
                )
            pooled_f = work_pool.tile([S_TOT, 2 * CK], f32, tag="pooled", name="pooled")
            nc.sync.dma_start(pooled_f[:], cc_out[:])

            # normalize: key part also folds in Ck**-0.5; cast to bf16
            keyval_bf = const_pool.tile([S_TOT, 2 * CK], bf16, tag="keyval", name="keyval")
            nc.vector.tensor_scalar(
                keyval_bf[:, :CK], pooled_f[:, :CK], iak_sb, None, ALU.mult
            )
            nc.vector.tensor_scalar(
                keyval_bf[:, CK:], pooled_f[:, CK:], iav_sb, None, ALU.mult
            )
            # key channel-major [128, S_TOT] x KK via PE transpose
            key_cm = const_pool.tile([128, KK * S_TOT], bf16, tag="keycm", name="keycm")
            for m in range(KK):
                kt = psmall.tile([128, S_TOT], bf16, tag="ps1", name="kt")
                nc.tensor.transpose(
                    kt[:, :S_TOT],
                    keyval_bf[:, m * 128:(m + 1) * 128],
                    ident[:S_TOT, :S_TOT],
                )
                nc.vector.tensor_copy(key_cm[:, m * S_TOT:(m + 1) * S_TOT], kt[:, :S_TOT])

            # ---- phase D: attention + fused output ----
            UPT = PIX_T // 128  # 4 pixel sub-blocks per tile
            for t in range(N_T):
                px = bass.ts(t, PIX_T)
                # sim for 4 sub-blocks packed into one PSUM bank [128, 440]
                sim = psmall.tile([128, UPT * S_TOT], f32, tag="ps1", name="sim")
                for u in range(UPT):
                    upx = bass.ts(t * UPT + u, 128)
                    for m in range(KK):
                        nc.tensor.matmul(
                            sim[:, u * S_TOT:(u + 1) * S_TOT], q_bf[m][:, upx],
                            key_cm[:, m * S_TOT:(m + 1) * S_TOT],
                            start=(m == 0), stop=(m == KK - 1),
                        )
                attn_f = attn_pool.tile([128, UPT * S_TOT], f32, tag="attnf", name="attnf")
                den = attn_pool.tile([128, UPT], f32, tag="den", name="den")
                for u in range(UPT):
                    usl = bass.ts(u, S_TOT)
                    nc.scalar.activation(
                        attn_f[:, usl], sim[:, usl], AF.Exp,
                        accum_out=den[:, u:u + 1],
                    )
                rden = attn_pool.tile([128, UPT], f32, tag="rden", name="rden")
                nc.vector.reciprocal(rden[:], den[:])
                attn_bf = attn_pool.tile([128, UPT * S_TOT], bf16, tag="attnb", name="attnb")
                aT = pacc.tile([S_TOT, PIX_T], bf16, tag="ps2", name="aT")
                for u in range(UPT):
                    usl = bass.ts(u, S_TOT)
                    nc.vector.tensor_scalar(
                        attn_bf[:, usl], attn_f[:, usl], rden[:, u:u + 1],
                        None, ALU.mult,
                    )
                    nc.tensor.transpose(
                        aT[:, u * 128:(u + 1) * 128], attn_bf[:, usl], ident[:]
                    )
                aT_sb = attn_pool.tile([S_TOT, PIX_T], bf16, tag="aTsb", name="aTsb")
                nc.vector.tensor_copy(aT_sb[:], aT[:])
                ctx_sb = [
                    out_pool.tile([128, PIX_T], bf16, tag=f"ctx{m}", name=f"ctx{m}", bufs=3)
                    for m in range(KK)
                ]
                for m in range(KK):
                    cp = pctx.tile([128, PIX_T], f32, tag="ps3", name="cp")
                    nc.tensor.matmul(
                        cp[:], keyval_bf[:, CK + m * 128:CK + (m + 1) * 128],
                        aT_sb[:],
                    )
                    nc.scalar.copy(ctx_sb[m][:], cp[:])
                # fused out: relu(w_co @ ctx + w_bfeat @ feats + b_co)
                for m in range(MO):
                    op = pbig.tile([128, PIX_T], f32, tag="pbig", name="pbig")
                    for k in range(KC):
                        nc.tensor.matmul(
                            op[:], wbf_sb[k][:, m * 128:(m + 1) * 128],
                            feats_bf[k][:, px], start=(k == 0), stop=False,
                        )
                    for k in range(KK):
                        nc.tensor.matmul(
                            op[:], wco_sb[k][:, m * 128:(m + 1) * 128],
                            ctx_sb[k][:], start=False, stop=(k == KK - 1),
                        )
                    o_sb = out_pool.tile([128, PIX_T], bf16, tag="osb", name="osb", bufs=4)
                    if m % 2 == 0:
                        nc.vector.tensor_scalar(
                            o_sb[:], op[:], bco_sb[m], 0.0, ALU.add, ALU.max
                        )
                    else:
                        nc.scalar.activation(
                            o_sb[:], op[:], AF.Relu, bias=bco_sb[m]
                        )
                    nc.sync.dma_start(out_d[m * 128:(m + 1) * 128, px], o_sb[:])

    nc.compile()
    return nc


_NC_CACHE = {}
TRACE = False
LAST_RESULT = {}


def kernel(feats, w_kq, b_kq, w_v, b_v, w_out, b_out, w_bot, b_bot):
    from concourse.bass_utils import run_bass_kernel_spmd

    feats = np.asarray(feats, dtype=np.float32)
    w_kq = np.asarray(w_kq, dtype=np.float32)
    b_kq = np.asarray(b_kq, dtype=np.float32)
    w_v = np.asarray(w_v, dtype=np.float32)
    b_v = np.asarray(b_v, dtype=np.float32)
    w_out = np.asarray(w_out, dtype=np.float32)
    b_out = np.asarray(b_out, dtype=np.float32)
    w_bot = np.asarray(w_bot, dtype=np.float32)
    b_bot = np.asarray(b_bot, dtype=np.float32)

    # host-side weight prep (tiny)
    KC, KK, MO = C // 128, CK // 128, CO // 128
    w_co = w_bot[:, :C] @ w_out                     # [CO, CK]
    b_co = w_bot[:, :C] @ b_out + b_bot             # [CO]
    w_bfeat = w_bot[:, C:]                          # [CO, C]
    wkqT = w_kq.T.reshape(KC, 128, CK)              # [k][p][m]
    wvT = w_v.T.reshape(KC, 128, CK)
    wbfT = w_bfeat.T.reshape(KC, 128, CO)
    wcoT = w_co.T.reshape(KK, 128, CO)
    w_qv = np.concatenate(
        [wkqT.transpose(1, 0, 2).reshape(128, KC * CK),
         wvT.transpose(1, 0, 2).reshape(128, KC * CK)], axis=1
    ).astype(BF16)
    w_ob = np.concatenate(
        [wbfT.transpose(1, 0, 2).reshape(128, KC * CO),
         wcoT.transpose(1, 0, 2).reshape(128, KK * CO)], axis=1
    ).astype(BF16)

    if "nc" not in _NC_CACHE:
        _NC_CACHE["nc"] = _build_kernel()
    nc = _NC_CACHE["nc"]

    sc = np.float32(1.0 / np.sqrt(CK))
    in_maps = []
    for core in range(N_CORES):
        b, half = core // 2, core % 2
        h0 = half * H_SH
        pmat, inv_area = _build_pool_mat(h0)
        biases = np.zeros((128, 10), dtype=np.float32)
        biases[:, 0:2] = b_kq.reshape(KK, 128).T
        biases[:, 2:4] = b_v.reshape(KK, 128).T
        biases[:, 4:8] = b_co.reshape(MO, 128).T
        biases[:S_TOT, 8] = inv_area * sc
        biases[:S_TOT, 9] = inv_area
        shard = np.ascontiguousarray(
            feats[b, :, h0:h0 + H_SH, :]
        ).reshape(C, NPIX).astype(BF16)
        in_maps.append(dict(
            feats=shard, w_qv=w_qv, w_ob=w_ob, biases=biases, pmat=pmat,
        ))

    res = run_bass_kernel_spmd(
        nc, in_maps, list(range(N_CORES)), trace=TRACE
    )
    LAST_RESULT["res"] = res

    out = np.empty((B, CO, H, W), dtype=np.float32)
    for core in range(N_CORES):
        b, half = core // 2, core % 2
        h0 = half * H_SH
        out[b, :, h0:h0 + H_SH, :] = np.asarray(
            res.results[core]["out"]
        ).astype(np.float32).reshape(CO, H_SH, W)
    return out



# revision 2
# speedup vs baseline: 1.4577x; 1.4577x over previous
"""APNB (asymmetric pyramid non-local block) Trainium2 kernel, fp8 edition.

Full inputs in, full output out. Sharding: 8 cores = (batch b=core//2,
H-half=core%2). Each core handles feats[b, :, 64*half:64*half+64, :]
(= [512, 8192] pixels).

All heavy GEMMs run as fp8-e4m3 DoubleRow matmuls (2 k-chunks per
instruction). The precision-critical bottleneck conv on feats uses a
3-term fp8 residual decomposition (w8@f8 + w8@fr8 + wr8@f8, shared
power-of-2 scale 2048) which matches bf16 accuracy. The attention side
(q/v projections, PPM pooling, softmax, pooled-value contraction with
W_out folded into W_bot on the host) is plain fp8 with per-tensor
power-of-2 scales. Final output is produced at 2048x scale and
exponent-shifted back on the host (exact).

Engine budget: elementwise drains/copies are load-balanced across
Vector/Scalar/GpSimd with a greedy cost tracker.
"""

import sys

for p in ("/opt/trn_rl_repo",):
    if p not in sys.path:
        sys.path.insert(0, p)

import numpy as np
import ml_dtypes

BF16 = ml_dtypes.bfloat16
F8 = ml_dtypes.float8_e4m3  # TRN float8e4 (max +-240)

# ---- problem constants (hardcoded per spec) ----
B, C, H, W = 4, 512, 128, 128
CK, CO = 256, 512
POOL_SCALES = (1, 3, 6, 8)
S_TOT = sum(s * s for s in POOL_SCALES)  # 110
S_HALF = S_TOT // 2                      # 55
N_CORES = 8
H_SH = H // 2          # 64 rows per core
NPIX = H_SH * W        # 8192 pixels per core
PIX_T = 512            # gemm pixel tile
N_T = NPIX // PIX_T    # 16
N_PAIR = H_SH // 2     # 32 row pairs for DoubleRow pooling

# power-of-2 scales
S_W = 64.0        # w_kq / w_v
S_Q = 64.0        # q8 = 64*relu(q), v8 = 64*v
S_KEY = 32.0      # keys
S_VP = 32.0       # pooled v (channel-major)
S_WCO = 64.0      # folded W_co
S_UP = 32.0       # u_pool
S_ATTN = 64.0     # attn weights
S_BF = 2048.0     # bottleneck feats weight + out psum scale
EXP_SCALE = 1.0 / (S_Q * S_KEY)   # fold q/key scales out inside exp
U_DRAIN = S_UP / (S_WCO * S_VP)   # u psum -> u8
OUT_UNSCALE = np.float32(1.0 / S_BF)


def _pool_bounds(n, s):
    i = np.arange(s)
    return (i * n) // s, -((-(i + 1) * n) // s)


def _build_pool_mat(h0):
    """P[w, r, bin] = 1 if global pixel (h0+r, w) is in bin; fp8 (exact)."""
    P = np.zeros((W, H_SH, S_TOT), dtype=np.float32)
    inv_area = np.zeros((S_TOT,), dtype=np.float32)
    off = 0
    for s in POOL_SCALES:
        hs, he = _pool_bounds(H, s)
        ws, we = _pool_bounds(W, s)
        for i in range(s):
            for j in range(s):
                b = off + i * s + j
                inv_area[b] = 1.0 / float((he[i] - hs[i]) * (we[j] - ws[j]))
                r0 = max(hs[i] - h0, 0)
                r1 = min(he[i] - h0, H_SH)
                if r1 > r0:
                    P[ws[j]:we[j], r0:r1, b] = 1.0
        off += s * s
    return P.astype(F8), inv_area


def _f8(x, scale=1.0):
    y = np.asarray(x, np.float32) * np.float32(scale)
    y = np.clip(y, -240.0, 240.0)
    return y.astype(F8)


def _build_kernel(sim_mode=False):
    import concourse.bass as bass
    import concourse.bacc as bacc
    import concourse.mybir as mybir
    from concourse import tile, masks

    dt = mybir.dt
    f32, bf16, f8 = dt.float32, dt.bfloat16, dt.float8e4
    AF = mybir.ActivationFunctionType
    ALU = mybir.AluOpType
    DR = mybir.MatmulPerfMode.DoubleRow

    nc = bacc.Bacc("TRN2", target_bir_lowering=False, debug=False)

    # ---- DRAM I/O ----
    # f8cat: [f8a | f8b | fr8a | fr8b], each [128, 2*NPIX] chunk-pair layout
    f8cat_d = nc.dram_tensor("f8cat", [128, 8 * NPIX], f8, kind="ExternalInput")
    wq8_d = nc.dram_tensor("wq8", [128, 2 * 512], f8, kind="ExternalInput")
    wv8_d = nc.dram_tensor("wv8", [128, 2 * 512], f8, kind="ExternalInput")
    wbf8_d = nc.dram_tensor("wbf8", [128, 2048], f8, kind="ExternalInput")
    wbfr8_d = nc.dram_tensor("wbfr8", [128, 2048], f8, kind="ExternalInput")
    wco8_d = nc.dram_tensor("wco8", [128, 1024], f8, kind="ExternalInput")
    pmat_d = nc.dram_tensor("pmat", [W, H_SH * S_TOT], f8, kind="ExternalInput")
    bias_d = nc.dram_tensor("biases", [128, 8], f32, kind="ExternalInput")
    out_d = nc.dram_tensor("out", [CO, NPIX], bf16, kind="ExternalOutput")

    # greedy engine load balancer for drains/copies
    class Rot:
        def __init__(self):
            self.load = {"dve": 0.0, "act": 0.0, "pool": 0.0}

        def pick(self, width, psum_in=True):
            cd = width * 1.05 + (130.0 if psum_in else 65.0)
            ca = width * 0.84 + 190.0
            cp = width * 1.39 + 125.0
            best = min(
                (self.load["dve"] + cd, cd, "dve"),
                (self.load["act"] + ca, ca, "act"),
                (self.load["pool"] + cp, cp, "pool"),
            )
            self.load[best[2]] = best[0]
            return best[2]

    rot = Rot()

    with tile.TileContext(nc) as tc:
        with (
            tc.tile_pool(name="const", bufs=1) as const_pool,
            tc.tile_pool(name="work", bufs=2) as work_pool,
            tc.tile_pool(name="outb", bufs=4) as out_pool,
            tc.tile_pool(name="pbig", bufs=3, space="PSUM") as pbig,
            tc.tile_pool(name="pacc", bufs=1, space="PSUM") as pacc,
            tc.tile_pool(name="prow", bufs=2, space="PSUM") as prow,
            tc.tile_pool(name="psim", bufs=1, space="PSUM") as psim,
            tc.tile_pool(name="paT", bufs=1, space="PSUM") as paT,
            tc.tile_pool(name="dram", bufs=1, space="DRAM") as dram_pool,
        ):
            def relu_drain(out, in_, bias_ap, width):
                e = rot.pick(width)
                if e == "dve":
                    nc.vector.tensor_scalar(out, in_, bias_ap, 0.0, ALU.add, ALU.max)
                elif e == "act":
                    nc.scalar.activation(out, in_, AF.Relu, bias=bias_ap)
                else:
                    nc.gpsimd.tensor_scalar(out, in_, bias_ap, 0.0, ALU.add, ALU.max)

            def copy_drain(out, in_, width):
                e = rot.pick(width)
                if e == "dve":
                    nc.vector.tensor_copy(out, in_)
                elif e == "act":
                    nc.scalar.copy(out, in_)
                else:
                    nc.gpsimd.tensor_copy(out, in_)

            # ---- constants / weights ----
            ident8 = const_pool.tile([128, 128], f8, tag="ident", name="ident8")
            masks.make_identity(nc, ident8[:])

            wq8_sb = const_pool.tile([128, 1024], f8, tag="wq8", name="wq8_sb")
            nc.sync.dma_start(wq8_sb[:], wq8_d[:])
            wv8_sb = const_pool.tile([128, 1024], f8, tag="wv8", name="wv8_sb")
            nc.sync.dma_start(wv8_sb[:], wv8_d[:])
            bias_sb = const_pool.tile([128, 8], f32, tag="bias", name="bias_sb")
            nc.sync.dma_start(bias_sb[:], bias_d[:])

            bkq_ap = [bias_sb[:, m:m + 1] for m in range(2)]            # 64*b_kq
            bco_ap = [bias_sb[:, 2 + m:3 + m] for m in range(4)]        # 2048*b_co
            iak_ap = bias_sb[:S_TOT, 6:7]                               # inv_area*sc/2
            iav_ap = bias_sb[:S_TOT, 7:8]                               # inv_area/2

            # feats fp8 (pair layouts) + residuals
            f8a = const_pool.tile([128, 2 * NPIX], f8, tag="f8a", name="f8a")
            f8b = const_pool.tile([128, 2 * NPIX], f8, tag="f8b", name="f8b")
            fr8a = const_pool.tile([128, 2 * NPIX], f8, tag="fr8a", name="fr8a")
            fr8b = const_pool.tile([128, 2 * NPIX], f8, tag="fr8b", name="fr8b")
            f8a3 = f8a[:].rearrange("k (i n) -> k i n", i=2)
            f8b3 = f8b[:].rearrange("k (i n) -> k i n", i=2)
            fr8a3 = fr8a[:].rearrange("k (i n) -> k i n", i=2)
            fr8b3 = fr8b[:].rearrange("k (i n) -> k i n", i=2)

            # q8 = 64*relu(q), v8 = 64*v; [128, 2*NPIX]: m-chunk planes
            q8 = const_pool.tile([128, 2 * NPIX], f8, tag="q8", name="q8")
            v8 = const_pool.tile([128, 2 * NPIX], f8, tag="v8", name="v8")
            q83 = q8[:].rearrange("k (i n) -> k i n", i=2)

            # DR weight APs
            def pair_ap(tile_ap, base, width):
                return tile_ap[:, base:base + 2 * width].rearrange(
                    "k (i m) -> k i m", i=2)

            wq_ap = [[pair_ap(wq8_sb[:], mc * 512 + p * 256, 128)
                      for p in range(2)] for mc in range(2)]
            wv_ap = [[pair_ap(wv8_sb[:], mc * 512 + p * 256, 128)
                      for p in range(2)] for mc in range(2)]

            # ---- phase A: q/v projections (fp8 DoubleRow) ----
            for t in range(N_T):
                px = bass.ts(t, PIX_T)
                if t % 2 == 0:
                    g = t // 2
                    c0 = g * 2 * PIX_T
                    nc.sync.dma_start(
                        f8a[:, c0:c0 + 1024], f8cat_d[:, c0:c0 + 1024])
                    nc.sync.dma_start(
                        f8a[:, NPIX + c0:NPIX + c0 + 1024],
                        f8cat_d[:, NPIX + c0:NPIX + c0 + 1024])
                    nc.sync.dma_start(
                        f8b[:, c0:c0 + 1024],
                        f8cat_d[:, 2 * NPIX + c0:2 * NPIX + c0 + 1024])
                    nc.sync.dma_start(
                        f8b[:, NPIX + c0:NPIX + c0 + 1024],
                        f8cat_d[:, 3 * NPIX + c0:3 * NPIX + c0 + 1024])
                for mc in range(2):
                    qp = pbig.tile([128, PIX_T], f32, tag="big", name="qp")
                    nc.tensor.matmul(qp[:], wq_ap[mc][0], f8a3[:, :, px],
                                     start=True, stop=False, perf_mode=DR)
                    nc.tensor.matmul(qp[:], wq_ap[mc][1], f8b3[:, :, px],
                                     start=False, stop=True, perf_mode=DR)
                    relu_drain(q8[:, mc * NPIX + t * PIX_T:
                                  mc * NPIX + (t + 1) * PIX_T],
                               qp[:], bkq_ap[mc], PIX_T)
                for mc in range(2):
                    vp = pbig.tile([128, PIX_T], f32, tag="big", name="vp")
                    nc.tensor.matmul(vp[:], wv_ap[mc][0], f8a3[:, :, px],
                                     start=True, stop=False, perf_mode=DR)
                    nc.tensor.matmul(vp[:], wv_ap[mc][1], f8b3[:, :, px],
                                     start=False, stop=True, perf_mode=DR)
                    copy_drain(v8[:, mc * NPIX + t * PIX_T:
                                  mc * NPIX + (t + 1) * PIX_T],
                               vp[:], PIX_T)

            # deferred const loads
            pmat_sb = const_pool.tile([W, H_SH * S_TOT], f8, tag="pmat", name="pmat_sb")
            nc.sync.dma_start(pmat_sb[:], pmat_d[:])
            pmat3 = pmat_sb[:].rearrange("w (r s) -> w r s", s=S_TOT)
            nc.sync.dma_start(fr8a[:], f8cat_d[:, 4 * NPIX:6 * NPIX])
            nc.sync.dma_start(fr8b[:], f8cat_d[:, 6 * NPIX:8 * NPIX])
            wbf8_sb = const_pool.tile([128, 2048], f8, tag="wbf8", name="wbf8_sb")
            nc.sync.dma_start(wbf8_sb[:], wbf8_d[:])
            wbfr8_sb = const_pool.tile([128, 2048], f8, tag="wbfr8", name="wbfr8_sb")
            nc.sync.dma_start(wbfr8_sb[:], wbfr8_d[:])
            wco8_sb = const_pool.tile([128, 1024], f8, tag="wco8", name="wco8_sb")
            nc.sync.dma_start(wco8_sb[:], wco8_d[:])

            wbf_ap = [[pair_ap(wbf8_sb[:], p * 1024 + 0, 512)[:, :, m * 128:(m + 1) * 128]
                       for p in range(2)] for m in range(4)]
            wbfr_ap = [[pair_ap(wbfr8_sb[:], p * 1024 + 0, 512)[:, :, m * 128:(m + 1) * 128]
                        for p in range(2)] for m in range(4)]
            wco_ap = [pair_ap(wco8_sb[:], 0, 512)[:, :, o * 128:(o + 1) * 128]
                      for o in range(4)]

            # ---- phase B: transpose q8/v8 rows, DoubleRow pooling ----
            pooled_ps = pacc.tile([S_TOT, 512], f32, tag="pooled", name="pooled_ps")
            qvT_prev = []
            for p in range(N_PAIR):
                rowT = prow.tile([128, 1024], f8, tag="rowT", name="rowT")
                for j in range(2):
                    r = 2 * p + j
                    rpx = bass.ts(r, 128)
                    for mc in range(2):
                        nc.tensor.transpose(
                            rowT[:, j * 512 + mc * 128:j * 512 + (mc + 1) * 128],
                            q8[:, mc * NPIX + r * 128:mc * NPIX + (r + 1) * 128],
                            ident8[:])
                        nc.tensor.transpose(
                            rowT[:, j * 512 + 256 + mc * 128:j * 512 + 256 + (mc + 1) * 128],
                            v8[:, mc * NPIX + r * 128:mc * NPIX + (r + 1) * 128],
                            ident8[:])
                qvT = work_pool.tile([128, 1024], f8, tag="qvT", name="qvT", bufs=3)
                copy_drain(qvT[:], rowT[:], 1024)
                qvT_prev.append(qvT)
                # emit pool matmul two pairs behind to cover copy latency
                if p >= 2:
                    pp = p - 2
                    nc.tensor.matmul(
                        pooled_ps[:], pmat3[:, 2 * pp:2 * pp + 2, :],
                        qvT_prev[pp][:].rearrange("w (i c) -> w i c", i=2),
                        start=(pp == 0), stop=False, perf_mode=DR)
            for pp in (N_PAIR - 2, N_PAIR - 1):
                nc.tensor.matmul(
                    pooled_ps[:], pmat3[:, 2 * pp:2 * pp + 2, :],
                    qvT_prev[pp][:].rearrange("w (i c) -> w i c", i=2),
                    start=False, stop=(pp == N_PAIR - 1), perf_mode=DR)

            # ---- phase C: AllReduce + pooled-side prep ----
            pooled_sb = work_pool.tile([S_TOT, 512], f32, tag="pooled", name="pooled_sb", bufs=1)
            nc.vector.tensor_copy(pooled_sb[:], pooled_ps[:])
            cc_in = dram_pool.tile([S_TOT, 512], f32, tag="cc_in", name="cc_in")
            cc_out = dram_pool.tile([S_TOT, 512], f32, tag="cc_out", name="cc_out")
            nc.sync.dma_start(cc_in[:], pooled_sb[:])
            if sim_mode:
                nc.sync.dma_start(cc_out[:], cc_in[:])
            else:
                nc.gpsimd.collective_compute(
                    "AllReduce",
                    ALU.add,
                    replica_groups=[[0, 1], [2, 3], [4, 5], [6, 7]],
                    ins=[cc_in.opt()],
                    outs=[cc_out.opt()],
                )
            pooled_f = work_pool.tile([S_TOT, 512], f32, tag="pooled", name="pooled_f", bufs=1)
            nc.sync.dma_start(pooled_f[:], cc_out[:])

            # keyval8: [110, 512] fp8 = [32*keys*sc | 32*v_pool]
            keyval8 = const_pool.tile([S_TOT, 512], f8, tag="keyval", name="keyval8")
            nc.vector.tensor_scalar(
                keyval8[:, :CK], pooled_f[:, :CK], iak_ap, None, ALU.mult)
            nc.vector.tensor_scalar(
                keyval8[:, CK:], pooled_f[:, CK:], iav_ap, None, ALU.mult)

            # key_cm / v_cm channel-major pair layouts [128, 2*110]
            kt = prow.tile([128, 2 * S_TOT], f8, tag="rowT", name="kt",
                           padded_shape=[128, 1024])
            for mc in range(2):
                nc.tensor.transpose(
                    kt[:, mc * S_TOT:(mc + 1) * S_TOT],
                    keyval8[:, mc * 128:(mc + 1) * 128],
                    ident8[:S_TOT, :S_TOT])
            key_cm = const_pool.tile([128, 2 * S_TOT], f8, tag="keycm", name="key_cm")
            nc.vector.tensor_copy(key_cm[:], kt[:])
            key3 = key_cm[:].rearrange("k (i s) -> k i s", i=2)

            vt = prow.tile([128, 2 * S_TOT], f8, tag="rowT", name="vt",
                           padded_shape=[128, 1024])
            for mc in range(2):
                nc.tensor.transpose(
                    vt[:, mc * S_TOT:(mc + 1) * S_TOT],
                    keyval8[:, CK + mc * 128:CK + (mc + 1) * 128],
                    ident8[:S_TOT, :S_TOT])
            v_cm = const_pool.tile([128, 2 * S_TOT], f8, tag="vcm", name="v_cm")
            nc.vector.tensor_copy(v_cm[:], vt[:])
            v3 = v_cm[:].rearrange("k (i s) -> k i s", i=2)

            # u_pool = W_co @ v_pool (DR), drained to fp8 at 32x
            u_ps = psim.tile([128, 4 * S_TOT], f32, tag="sim", name="u_ps",
                             padded_shape=[128, 512])
            for o in range(4):
                nc.tensor.matmul(u_ps[:, o * S_TOT:(o + 1) * S_TOT],
                                 wco_ap[o], v3, start=True, stop=True,
                                 perf_mode=DR)
            u_sb = const_pool.tile([128, 4 * S_TOT], f8, tag="usb", name="u_sb")
            nc.vector.tensor_scalar(u_sb[:], u_ps[:], U_DRAIN, None, ALU.mult)

            # u_poolT in [55, 2, 128] DR layout per o-chunk
            upT_ps = paT.tile([S_HALF, 4 * 256], f8, tag="aT", name="upT_ps",
                              padded_shape=[128, 1024])
            for o in range(4):
                for h in range(2):
                    nc.tensor.transpose(
                        upT_ps[:, o * 256 + h * 128:o * 256 + (h + 1) * 128],
                        u_sb[:, o * S_TOT + h * S_HALF:o * S_TOT + (h + 1) * S_HALF],
                        ident8[:])
            upT_sb = const_pool.tile([S_HALF, 4 * 256], f8, tag="upT", name="upT_sb")
            nc.vector.tensor_copy(upT_sb[:], upT_ps[:])
            upT_ap = [upT_sb[:, o * 256:(o + 1) * 256].rearrange(
                "s (i m) -> s i m", i=2) for o in range(4)]

            # ---- phase D: attention + fused output (software pipelined) ----
            def emit_sim_chain(t):
                sim = psim.tile([128, 4 * S_TOT], f32, tag="sim", name="sim",
                                padded_shape=[128, 512])
                for u in range(4):
                    upx = bass.ts(t * 4 + u, 128)
                    nc.tensor.matmul(
                        sim[:, u * S_TOT:(u + 1) * S_TOT],
                        q83[:, :, upx], key3, start=True, stop=True,
                        perf_mode=DR)
                attn_f = work_pool.tile([128, 4 * S_TOT], f32, tag="attnf", name="attn_f")
                nc.scalar.activation(attn_f[:], sim[:], AF.Exp, scale=EXP_SCALE)
                den = work_pool.tile([128, 4], f32, tag="den", name="den")
                nc.vector.tensor_reduce(
                    den[:], attn_f[:].rearrange("p (u s) -> p u s", s=S_TOT),
                    axis=mybir.AxisListType.X, op=ALU.add)
                rden = work_pool.tile([128, 4], f32, tag="rden", name="rden")
                nc.vector.reciprocal(rden[:], den[:])
                attn8 = work_pool.tile([128, 4 * S_TOT], f8, tag="attn8", name="attn8")
                for u in range(4):
                    usl = bass.ts(u, S_TOT)
                    nc.vector.tensor_scalar(
                        attn8[:, usl], attn_f[:, usl], rden[:, u:u + 1],
                        S_ATTN, ALU.mult, ALU.mult)
                return attn8

            def emit_attnT(t, attn8):
                aTp = paT.tile([S_HALF, 1024], f8, tag="aT", name="aTp",
                               padded_shape=[128, 1024])
                for u in range(4):
                    for h in range(2):
                        nc.tensor.transpose(
                            aTp[:, h * 512 + u * 128:h * 512 + (u + 1) * 128],
                            attn8[:, u * S_TOT + h * S_HALF:
                                  u * S_TOT + (h + 1) * S_HALF],
                            ident8[:])
                aT_sb = work_pool.tile([S_HALF, 1024], f8, tag="aTsb", name="aT_sb")
                copy_drain(aT_sb[:], aTp[:], 1024)
                return aT_sb

            def emit_out_block(t, aT_sb):
                px = bass.ts(t, PIX_T)
                aT3 = aT_sb[:].rearrange("s (i n) -> s i n", i=2)
                for m in range(4):
                    op = pbig.tile([128, PIX_T], f32, tag="big", name="op")
                    nc.tensor.matmul(op[:], wbf_ap[m][0], f8a3[:, :, px],
                                     start=True, stop=False, perf_mode=DR)
                    nc.tensor.matmul(op[:], wbf_ap[m][1], f8b3[:, :, px],
                                     start=False, stop=False, perf_mode=DR)
                    nc.tensor.matmul(op[:], wbfr_ap[m][0], f8a3[:, :, px],
                                     start=False, stop=False, perf_mode=DR)
                    nc.tensor.matmul(op[:], wbfr_ap[m][1], f8b3[:, :, px],
                                     start=False, stop=False, perf_mode=DR)
                    nc.tensor.matmul(op[:], wbf_ap[m][0], fr8a3[:, :, px],
                                     start=False, stop=False, perf_mode=DR)
                    nc.tensor.matmul(op[:], wbf_ap[m][1], fr8b3[:, :, px],
                                     start=False, stop=False, perf_mode=DR)
                    nc.tensor.matmul(op[:], upT_ap[m], aT3,
                                     start=False, stop=True, perf_mode=DR)
                    o_sb = out_pool.tile([128, PIX_T], bf16, tag="osb", name="o_sb")
                    relu_drain(o_sb[:], op[:], bco_ap[m], PIX_T)
                    nc.sync.dma_start(out_d[m * 128:(m + 1) * 128, px], o_sb[:])

            prev = None
            for t in range(N_T):
                attn8 = emit_sim_chain(t)
                if prev is not None:
                    emit_out_block(prev[0], prev[1])
                aT_sb = emit_attnT(t, attn8)
                prev = (t, aT_sb)
            emit_out_block(prev[0], prev[1])

    nc.compile()
    return nc


_NC_CACHE = {}
TRACE = False
LAST_RESULT = {}


def kernel(feats, w_kq, b_kq, w_v, b_v, w_out, b_out, w_bot, b_bot):
    from concourse.bass_utils import run_bass_kernel_spmd

    feats = np.asarray(feats, dtype=np.float32)
    w_kq = np.asarray(w_kq, dtype=np.float32)
    b_kq = np.asarray(b_kq, dtype=np.float32)
    w_v = np.asarray(w_v, dtype=np.float32)
    b_v = np.asarray(b_v, dtype=np.float32)
    w_out = np.asarray(w_out, dtype=np.float32)
    b_out = np.asarray(b_out, dtype=np.float32)
    w_bot = np.asarray(w_bot, dtype=np.float32)
    b_bot = np.asarray(b_bot, dtype=np.float32)

    # host-side weight prep
    w_co = w_bot[:, :C] @ w_out                     # [CO, CK]
    b_co = w_bot[:, :C] @ b_out + b_bot + w_co @ b_v
    w_bf = w_bot[:, C:]                             # [CO, C]

    # wq8/wv8: [k, mc*512 + p*256 + i*128 + m] = 64*w[mc*128+m, (2p+i)*128+k]
    def proj_pack(wm):
        arr = wm.T.reshape(2, 2, 128, 2, 128)       # [p, i, k, mc, m]
        return _f8(arr.transpose(2, 3, 0, 1, 4).reshape(128, 1024), S_W)

    wq8 = proj_pack(w_kq)
    wv8 = proj_pack(w_v)

    # wbf8: [k, p*1024 + i*512 + o] = 2048*w_bf[o, (2p+i)*128+k]
    arr = w_bf.T.reshape(2, 2, 128, CO)             # [p, i, k, o]
    wbf8 = _f8(arr.transpose(2, 0, 1, 3).reshape(128, 2048), S_BF)
    wbf_resid = arr.transpose(2, 0, 1, 3).reshape(128, 2048) - \
        wbf8.astype(np.float32) / np.float32(S_BF)
    wbfr8 = _f8(wbf_resid, S_BF)

    # wco8: [k, i*512 + o] = 64*w_co[o, i*128+k]
    arr = w_co.T.reshape(2, 128, CO)                # [i, k, o]
    wco8 = _f8(arr.transpose(1, 0, 2).reshape(128, 1024), S_WCO)

    if "nc" not in _NC_CACHE:
        _NC_CACHE["nc"] = _build_kernel()
    nc = _NC_CACHE["nc"]

    sc = np.float32(1.0 / np.sqrt(CK))
    in_maps = []
    for core in range(N_CORES):
        b, half = core // 2, core % 2
        h0 = half * H_SH
        pmat, inv_area = _build_pool_mat(h0)
        biases = np.zeros((128, 8), dtype=np.float32)
        biases[:, 0:2] = (S_W * b_kq).reshape(2, 128).T
        biases[:, 2:6] = (S_BF * b_co).reshape(4, 128).T
        biases[:S_TOT, 6] = inv_area * sc * (S_KEY / S_Q)
        biases[:S_TOT, 7] = inv_area * (S_VP / S_Q)
        shard = np.ascontiguousarray(
            feats[b, :, h0:h0 + H_SH, :]).reshape(C, NPIX)
        f8v = _f8(shard).astype(np.float32)
        fr8v = _f8(shard - f8v)
        f8v = f8v.astype(F8)
        # pair layouts [128, 2*NPIX]: chunk-pair planes side by side
        c4 = f8v.reshape(4, 128, NPIX)
        r4 = fr8v.reshape(4, 128, NPIX)
        f8cat = np.concatenate(
            [np.concatenate([c4[0], c4[1]], axis=1),
             np.concatenate([c4[2], c4[3]], axis=1),
             np.concatenate([r4[0], r4[1]], axis=1),
             np.concatenate([r4[2], r4[3]], axis=1)], axis=1)
        in_maps.append(dict(
            f8cat=np.ascontiguousarray(f8cat), wq8=wq8, wv8=wv8,
            wbf8=wbf8, wbfr8=wbfr8, wco8=wco8,
            pmat=pmat.reshape(W, H_SH * S_TOT), biases=biases,
        ))

    res = run_bass_kernel_spmd(
        nc, in_maps, list(range(N_CORES)), trace=TRACE
    )
    LAST_RESULT["res"] = res

    out = np.empty((B, CO, H, W), dtype=np.float32)
    for core in range(N_CORES):
        b, half = core // 2, core % 2
        h0 = half * H_SH
        out[b, :, h0:h0 + H_SH, :] = (
            np.asarray(res.results[core]["out"]).astype(np.float32)
            * OUT_UNSCALE
        ).reshape(CO, H_SH, W)
    return out


# revision 3
# speedup vs baseline: 1.5562x; 1.0676x over previous
"""APNB (asymmetric pyramid non-local block) Trainium2 kernel, fp8 edition.

Full inputs in, full output out. Sharding: 8 cores = (batch b=core//2,
H-half=core%2). Each core handles feats[b, :, 64*half:64*half+64, :]
(= [512, 8192] pixels).

All heavy GEMMs run as fp8-e4m3 DoubleRow matmuls (2 k-chunks per
instruction). The precision-critical bottleneck conv on feats uses a
3-term fp8 residual decomposition (w8@f8 + w8@fr8 + wr8@f8, shared
power-of-2 scale 2048) which matches bf16 accuracy. The attention side
(q/v projections, PPM pooling, softmax, pooled-value contraction with
W_out folded into W_bot on the host) is plain fp8 with per-tensor
power-of-2 scales. Final output is produced at 2048x scale and
exponent-shifted back on the host (exact).

Engine budget: elementwise drains/copies are load-balanced across
Vector/Scalar/GpSimd with a greedy cost tracker; phases A/B are
interleaved and phase D is software-pipelined so the PE never starves.
"""

import sys

for p in ("/opt/trn_rl_repo",):
    if p not in sys.path:
        sys.path.insert(0, p)

import numpy as np
import ml_dtypes

BF16 = ml_dtypes.bfloat16
F8 = ml_dtypes.float8_e4m3  # TRN float8e4 (max +-240)

# ---- problem constants (hardcoded per spec) ----
B, C, H, W = 4, 512, 128, 128
CK, CO = 256, 512
POOL_SCALES = (1, 3, 6, 8)
S_TOT = sum(s * s for s in POOL_SCALES)  # 110
S_HALF = S_TOT // 2                      # 55
N_CORES = 8
H_SH = H // 2          # 64 rows per core
NPIX = H_SH * W        # 8192 pixels per core
PIX_T = 512            # gemm pixel tile
N_T = NPIX // PIX_T    # 16
N_PAIR = H_SH // 2     # 32 row pairs for DoubleRow pooling

# power-of-2 scales
S_W = 64.0        # w_kq / w_v
S_Q = 64.0        # q8 = 64*relu(q), v8 = 64*v
S_KEY = 32.0      # keys
S_VP = 32.0       # pooled v (channel-major)
S_WCO = 64.0      # folded W_co
S_UP = 32.0       # u_pool
S_ATTN = 64.0     # attn weights
S_BF = 2048.0     # bottleneck feats weight + out psum scale
EXP_SCALE = 1.0 / (S_Q * S_KEY)   # fold q/key scales out inside exp
U_DRAIN = S_UP / (S_WCO * S_VP)   # u psum -> u8
OUT_UNSCALE = np.float32(1.0 / S_BF)


def _pool_bounds(n, s):
    i = np.arange(s)
    return (i * n) // s, -((-(i + 1) * n) // s)


def _build_pool_mat(h0):
    """P[w, r, bin] = 1 if global pixel (h0+r, w) is in bin; fp8 (exact)."""
    P = np.zeros((W, H_SH, S_TOT), dtype=np.float32)
    inv_area = np.zeros((S_TOT,), dtype=np.float32)
    off = 0
    for s in POOL_SCALES:
        hs, he = _pool_bounds(H, s)
        ws, we = _pool_bounds(W, s)
        for i in range(s):
            for j in range(s):
                b = off + i * s + j
                inv_area[b] = 1.0 / float((he[i] - hs[i]) * (we[j] - ws[j]))
                r0 = max(hs[i] - h0, 0)
                r1 = min(he[i] - h0, H_SH)
                if r1 > r0:
                    P[ws[j]:we[j], r0:r1, b] = 1.0
        off += s * s
    return P.astype(F8), inv_area


def _f8(x, scale=1.0):
    y = np.asarray(x, np.float32) * np.float32(scale)
    y = np.clip(y, -240.0, 240.0)
    return y.astype(F8)


def _build_kernel(sim_mode=False):
    import concourse.bass as bass
    import concourse.bacc as bacc
    import concourse.mybir as mybir
    from concourse import tile, masks

    dt = mybir.dt
    f32, bf16, f8 = dt.float32, dt.bfloat16, dt.float8e4
    AF = mybir.ActivationFunctionType
    ALU = mybir.AluOpType
    DR = mybir.MatmulPerfMode.DoubleRow

    nc = bacc.Bacc("TRN2", target_bir_lowering=False, debug=False)

    # ---- DRAM I/O ----
    # f8cat: [f8a | f8b | fr8a | fr8b], each [128, 2*NPIX] chunk-pair layout
    f8cat_d = nc.dram_tensor("f8cat", [128, 8 * NPIX], f8, kind="ExternalInput")
    wq8_d = nc.dram_tensor("wq8", [128, 2 * 512], f8, kind="ExternalInput")
    wv8_d = nc.dram_tensor("wv8", [128, 2 * 512], f8, kind="ExternalInput")
    wbf8_d = nc.dram_tensor("wbf8", [128, 2048], f8, kind="ExternalInput")
    wbfr8_d = nc.dram_tensor("wbfr8", [128, 2048], f8, kind="ExternalInput")
    wco8_d = nc.dram_tensor("wco8", [128, 1024], f8, kind="ExternalInput")
    pmat_d = nc.dram_tensor("pmat", [W, H_SH * S_TOT], f8, kind="ExternalInput")
    bias_d = nc.dram_tensor("biases", [128, 8], f32, kind="ExternalInput")
    out_d = nc.dram_tensor("out", [CO, NPIX], bf16, kind="ExternalOutput")

    # greedy engine load balancer for drains/copies
    class Rot:
        def __init__(self):
            self.load = {"dve": 0.0, "act": 0.0, "pool": 0.0}

        def add(self, e, ns):
            self.load[e] += ns

        def pick(self, width, psum_in=True):
            cd = width * 1.05 + (130.0 if psum_in else 65.0)
            ca = width * 0.84 + 190.0
            cp = width * 1.39 + 125.0
            best = min(
                (self.load["dve"] + cd, cd, "dve"),
                (self.load["act"] + ca, ca, "act"),
                (self.load["pool"] + cp, cp, "pool"),
            )
            self.load[best[2]] = best[0]
            return best[2]

    rot = Rot()

    with tile.TileContext(nc) as tc:
        with (
            tc.tile_pool(name="const", bufs=1) as const_pool,
            tc.tile_pool(name="work", bufs=2) as work_pool,
            tc.tile_pool(name="outb", bufs=4) as out_pool,
            tc.tile_pool(name="pbig", bufs=4, space="PSUM") as pbig,
            tc.tile_pool(name="pacc", bufs=1, space="PSUM") as pacc,
            tc.tile_pool(name="p1k", bufs=2, space="PSUM") as p1k,
            tc.tile_pool(name="psim", bufs=1, space="PSUM") as psim,
            tc.tile_pool(name="dram", bufs=1, space="DRAM") as dram_pool,
        ):
            def relu_drain(out, in_, bias_ap, width):
                e = rot.pick(width)
                if e == "dve":
                    nc.vector.tensor_scalar(out, in_, bias_ap, 0.0, ALU.add, ALU.max)
                elif e == "act":
                    nc.scalar.activation(out, in_, AF.Relu, bias=bias_ap)
                else:
                    nc.gpsimd.tensor_scalar(out, in_, bias_ap, 0.0, ALU.add, ALU.max)

            def copy_drain(out, in_, width):
                e = rot.pick(width)
                if e == "dve":
                    nc.vector.tensor_copy(out, in_)
                elif e == "act":
                    nc.scalar.copy(out, in_)
                else:
                    nc.gpsimd.tensor_copy(out, in_)

            # ---- constants / weights ----
            ident8 = const_pool.tile([128, 128], f8, tag="ident", name="ident8")
            masks.make_identity(nc, ident8[:])

            # feats fp8 (pair layouts) + residuals
            f8a = const_pool.tile([128, 2 * NPIX], f8, tag="f8a", name="f8a")
            f8b = const_pool.tile([128, 2 * NPIX], f8, tag="f8b", name="f8b")
            fr8a = const_pool.tile([128, 2 * NPIX], f8, tag="fr8a", name="fr8a")
            fr8b = const_pool.tile([128, 2 * NPIX], f8, tag="fr8b", name="fr8b")
            f8a3 = f8a[:].rearrange("k (i n) -> k i n", i=2)
            f8b3 = f8b[:].rearrange("k (i n) -> k i n", i=2)
            fr8a3 = fr8a[:].rearrange("k (i n) -> k i n", i=2)
            fr8b3 = fr8b[:].rearrange("k (i n) -> k i n", i=2)

            wq8_sb = const_pool.tile([128, 1024], f8, tag="wq8", name="wq8_sb")
            wv8_sb = const_pool.tile([128, 1024], f8, tag="wv8", name="wv8_sb")
            bias_sb = const_pool.tile([128, 8], f32, tag="bias", name="bias_sb")
            pmat_sb = const_pool.tile([W, H_SH * S_TOT], f8, tag="pmat", name="pmat_sb")
            wbf8_sb = const_pool.tile([128, 2048], f8, tag="wbf8", name="wbf8_sb")
            wbfr8_sb = const_pool.tile([128, 2048], f8, tag="wbfr8", name="wbfr8_sb")
            wco8_sb = const_pool.tile([128, 1024], f8, tag="wco8", name="wco8_sb")

            def dma_f8_slices(t):
                g = t // 2
                c0 = g * 2 * PIX_T
                nc.sync.dma_start(f8a[:, c0:c0 + 1024], f8cat_d[:, c0:c0 + 1024])
                nc.sync.dma_start(
                    f8a[:, NPIX + c0:NPIX + c0 + 1024],
                    f8cat_d[:, NPIX + c0:NPIX + c0 + 1024])
                nc.sync.dma_start(
                    f8b[:, c0:c0 + 1024],
                    f8cat_d[:, 2 * NPIX + c0:2 * NPIX + c0 + 1024])
                nc.sync.dma_start(
                    f8b[:, NPIX + c0:NPIX + c0 + 1024],
                    f8cat_d[:, 3 * NPIX + c0:3 * NPIX + c0 + 1024])

            # first feats slices, then small consts
            dma_f8_slices(0)
            nc.sync.dma_start(wq8_sb[:], wq8_d[:])
            nc.sync.dma_start(wv8_sb[:], wv8_d[:])
            nc.sync.dma_start(bias_sb[:], bias_d[:])

            bkq_ap = [bias_sb[:, m:m + 1] for m in range(2)]            # 64*b_kq
            bco_ap = [bias_sb[:, 2 + m:3 + m] for m in range(4)]        # 2048*b_co
            iak_ap = bias_sb[:S_TOT, 6:7]                               # inv_area*sc/2
            iav_ap = bias_sb[:S_TOT, 7:8]                               # inv_area/2

            pmat3 = pmat_sb[:].rearrange("w (r s) -> w r s", s=S_TOT)

            # q8 = 64*relu(q), v8 = 64*v; [128, 2*NPIX]: m-chunk planes
            q8 = const_pool.tile([128, 2 * NPIX], f8, tag="q8", name="q8")
            v8 = const_pool.tile([128, 2 * NPIX], f8, tag="v8", name="v8")
            q83 = q8[:].rearrange("k (i n) -> k i n", i=2)

            # DR weight APs
            def pair_ap(tile_ap, base, width):
                return tile_ap[:, base:base + 2 * width].rearrange(
                    "k (i m) -> k i m", i=2)

            wq_ap = [[pair_ap(wq8_sb[:], mc * 512 + p * 256, 128)
                      for p in range(2)] for mc in range(2)]
            wv_ap = [[pair_ap(wv8_sb[:], mc * 512 + p * 256, 128)
                      for p in range(2)] for mc in range(2)]
            wbf_ap = [[pair_ap(wbf8_sb[:], p * 1024, 512)[:, :, m * 128:(m + 1) * 128]
                       for p in range(2)] for m in range(4)]
            wbfr_ap = [[pair_ap(wbfr8_sb[:], p * 1024, 512)[:, :, m * 128:(m + 1) * 128]
                        for p in range(2)] for m in range(4)]
            wco_ap = [pair_ap(wco8_sb[:], 0, 512)[:, :, o * 128:(o + 1) * 128]
                      for o in range(4)]

            # ---- phases A+B interleaved ----
            pooled_ps = pacc.tile([S_TOT, 512], f32, tag="pooled", name="pooled_ps")
            qvT_list = []

            def emit_a_tile(t):
                px = bass.ts(t, PIX_T)
                for mc in range(2):
                    qp = pbig.tile([128, PIX_T], f32, tag="big", name="qp")
                    nc.tensor.matmul(qp[:], wq_ap[mc][0], f8a3[:, :, px],
                                     start=True, stop=False, perf_mode=DR)
                    nc.tensor.matmul(qp[:], wq_ap[mc][1], f8b3[:, :, px],
                                     start=False, stop=True, perf_mode=DR)
                    relu_drain(q8[:, mc * NPIX + t * PIX_T:
                                  mc * NPIX + (t + 1) * PIX_T],
                               qp[:], bkq_ap[mc], PIX_T)
                for mc in range(2):
                    vp = pbig.tile([128, PIX_T], f32, tag="big", name="vp")
                    nc.tensor.matmul(vp[:], wv_ap[mc][0], f8a3[:, :, px],
                                     start=True, stop=False, perf_mode=DR)
                    nc.tensor.matmul(vp[:], wv_ap[mc][1], f8b3[:, :, px],
                                     start=False, stop=True, perf_mode=DR)
                    copy_drain(v8[:, mc * NPIX + t * PIX_T:
                                  mc * NPIX + (t + 1) * PIX_T],
                               vp[:], PIX_T)

            def emit_b_pair(p):
                rowT = p1k.tile([128, 1024], f8, tag="t1k", name="rowT")
                for j in range(2):
                    r = 2 * p + j
                    for mc in range(2):
                        nc.tensor.transpose(
                            rowT[:, j * 512 + mc * 128:j * 512 + (mc + 1) * 128],
                            q8[:, mc * NPIX + r * 128:mc * NPIX + (r + 1) * 128],
                            ident8[:])
                        nc.tensor.transpose(
                            rowT[:, j * 512 + 256 + mc * 128:j * 512 + 256 + (mc + 1) * 128],
                            v8[:, mc * NPIX + r * 128:mc * NPIX + (r + 1) * 128],
                            ident8[:])
                qvT = work_pool.tile([128, 1024], f8, tag="qvT", name="qvT", bufs=3)
                copy_drain(qvT[:], rowT[:], 1024)
                qvT_list.append(qvT)

            def emit_pool_mm(pp):
                nc.tensor.matmul(
                    pooled_ps[:], pmat3[:, 2 * pp:2 * pp + 2, :],
                    qvT_list[pp][:].rearrange("w (i c) -> w i c", i=2),
                    start=(pp == 0), stop=(pp == N_PAIR - 1), perf_mode=DR)

            for t in range(N_T):
                if t >= 2 and t % 2 == 0:
                    dma_f8_slices(t)
                if 1 <= t <= 4:   # pmat in 4 row-chunks (16 rows each)
                    g = t - 1
                    nc.sync.dma_start(
                        pmat_sb[:, g * 16 * S_TOT:(g + 1) * 16 * S_TOT],
                        pmat_d[:, g * 16 * S_TOT:(g + 1) * 16 * S_TOT])
                if 5 <= t <= 12:  # fr8 in 8 column-chunks
                    g = t - 5
                    base = 4 * NPIX + g * 2048
                    dst = fr8a if g < 4 else fr8b
                    off = (g % 4) * 2048
                    nc.sync.dma_start(dst[:, off:off + 2048],
                                      f8cat_d[:, base:base + 2048])
                if t == 13:
                    nc.sync.dma_start(wbf8_sb[:], wbf8_d[:])
                if t == 14:
                    nc.sync.dma_start(wbfr8_sb[:], wbfr8_d[:])
                if t == 15:
                    nc.sync.dma_start(wco8_sb[:], wco8_d[:])
                emit_a_tile(t)
                if t >= 1:
                    for p in (2 * (t - 1), 2 * (t - 1) + 1):
                        emit_b_pair(p)
                        if p >= 2:
                            emit_pool_mm(p - 2)
            for p in (2 * (N_T - 1), 2 * (N_T - 1) + 1):
                emit_b_pair(p)
            for pp in range(N_PAIR - 4, N_PAIR):
                emit_pool_mm(pp)

            # ---- phase C: AllReduce + pooled-side prep ----
            pooled_sb = work_pool.tile([S_TOT, 512], f32, tag="pooled", name="pooled_sb", bufs=1)
            nc.vector.tensor_copy(pooled_sb[:], pooled_ps[:])
            rot.add("dve", 700)
            cc_in = dram_pool.tile([S_TOT, 512], f32, tag="cc_in", name="cc_in")
            cc_out = dram_pool.tile([S_TOT, 512], f32, tag="cc_out", name="cc_out")
            nc.sync.dma_start(cc_in[:], pooled_sb[:])
            if sim_mode:
                nc.sync.dma_start(cc_out[:], cc_in[:])
            else:
                nc.gpsimd.collective_compute(
                    "AllReduce",
                    ALU.add,
                    replica_groups=[[0, 1], [2, 3], [4, 5], [6, 7]],
                    ins=[cc_in.opt()],
                    outs=[cc_out.opt()],
                )
            pooled_f = work_pool.tile([S_TOT, 512], f32, tag="pooled", name="pooled_f", bufs=1)
            nc.sync.dma_start(pooled_f[:], cc_out[:])

            # keyval8: [110, 512] fp8 = [32*keys*sc | 32*v_pool]
            keyval8 = const_pool.tile([S_TOT, 512], f8, tag="keyval", name="keyval8")
            nc.vector.tensor_scalar(
                keyval8[:, :CK], pooled_f[:, :CK], iak_ap, None, ALU.mult)
            nc.vector.tensor_scalar(
                keyval8[:, CK:], pooled_f[:, CK:], iav_ap, None, ALU.mult)
            rot.add("dve", 700)

            # key_cm / v_cm channel-major pair layouts [128, 2*110]
            kt = p1k.tile([128, 2 * S_TOT], f8, tag="t1k", name="kt",
                          padded_shape=[128, 1024])
            for mc in range(2):
                nc.tensor.transpose(
                    kt[:, mc * S_TOT:(mc + 1) * S_TOT],
                    keyval8[:, mc * 128:(mc + 1) * 128],
                    ident8[:S_TOT, :S_TOT])
            key_cm = const_pool.tile([128, 2 * S_TOT], f8, tag="keycm", name="key_cm")
            nc.scalar.copy(key_cm[:], kt[:])
            rot.add("act", 400)
            key3 = key_cm[:].rearrange("k (i s) -> k i s", i=2)

            vt = p1k.tile([128, 2 * S_TOT], f8, tag="t1k", name="vt",
                          padded_shape=[128, 1024])
            for mc in range(2):
                nc.tensor.transpose(
                    vt[:, mc * S_TOT:(mc + 1) * S_TOT],
                    keyval8[:, CK + mc * 128:CK + (mc + 1) * 128],
                    ident8[:S_TOT, :S_TOT])
            v_cm = const_pool.tile([128, 2 * S_TOT], f8, tag="vcm", name="v_cm")
            nc.scalar.copy(v_cm[:], vt[:])
            rot.add("act", 400)
            v3 = v_cm[:].rearrange("k (i s) -> k i s", i=2)

            # u_pool = W_co @ v_pool (DR), drained to fp8 at 32x
            u_ps = psim.tile([128, 4 * S_TOT], f32, tag="sim", name="u_ps",
                             padded_shape=[128, 512])
            for o in range(4):
                nc.tensor.matmul(u_ps[:, o * S_TOT:(o + 1) * S_TOT],
                                 wco_ap[o], v3, start=True, stop=True,
                                 perf_mode=DR)
            u_sb = const_pool.tile([128, 4 * S_TOT], f8, tag="usb", name="u_sb")
            nc.vector.tensor_scalar(u_sb[:], u_ps[:], U_DRAIN, None, ALU.mult)
            rot.add("dve", 600)

            # u_poolT in [55, 2, 128] DR layout per o-chunk
            upT_ps = p1k.tile([S_HALF, 4 * 256], f8, tag="t1k", name="upT_ps",
                              padded_shape=[128, 1024])
            for o in range(4):
                for h in range(2):
                    nc.tensor.transpose(
                        upT_ps[:, o * 256 + h * 128:o * 256 + (h + 1) * 128],
                        u_sb[:, o * S_TOT + h * S_HALF:o * S_TOT + (h + 1) * S_HALF],
                        ident8[:])
            upT_sb = const_pool.tile([S_HALF, 4 * 256], f8, tag="upT", name="upT_sb")
            nc.scalar.copy(upT_sb[:], upT_ps[:])
            rot.add("act", 1000)
            upT_ap = [upT_sb[:, o * 256:(o + 1) * 256].rearrange(
                "s (i m) -> s i m", i=2) for o in range(4)]

            # ---- phase D: attention + fused output (software pipelined) ----
            def emit_sim(t):
                sim = psim.tile([128, 4 * S_TOT], f32, tag="sim", name="sim",
                                padded_shape=[128, 512])
                for u in range(4):
                    upx = bass.ts(t * 4 + u, 128)
                    nc.tensor.matmul(
                        sim[:, u * S_TOT:(u + 1) * S_TOT],
                        q83[:, :, upx], key3, start=True, stop=True,
                        perf_mode=DR)
                attn_f = work_pool.tile([128, 4 * S_TOT], f32, tag="attnf", name="attn_f")
                nc.scalar.activation(attn_f[:], sim[:], AF.Exp, scale=EXP_SCALE)
                rot.add("act", 750)
                den = work_pool.tile([128, 4], f32, tag="den", name="den")
                nc.vector.tensor_reduce(
                    den[:], attn_f[:].rearrange("p (u s) -> p u s", s=S_TOT),
                    axis=mybir.AxisListType.X, op=ALU.add)
                rden = work_pool.tile([128, 4], f32, tag="rden", name="rden")
                nc.vector.reciprocal(rden[:], den[:])
                attn8 = work_pool.tile([128, 4 * S_TOT], f8, tag="attn8", name="attn8")
                for u in range(4):
                    usl = bass.ts(u, S_TOT)
                    nc.vector.tensor_scalar(
                        attn8[:, usl], attn_f[:, usl], rden[:, u:u + 1],
                        S_ATTN, ALU.mult, ALU.mult)
                rot.add("dve", 2000)
                return attn8

            def emit_bf(t, m, op):
                px = bass.ts(t, PIX_T)
                nc.tensor.matmul(op[:], wbf_ap[m][0], f8a3[:, :, px],
                                 start=True, stop=False, perf_mode=DR)
                nc.tensor.matmul(op[:], wbf_ap[m][1], f8b3[:, :, px],
                                 start=False, stop=False, perf_mode=DR)
                nc.tensor.matmul(op[:], wbfr_ap[m][0], f8a3[:, :, px],
                                 start=False, stop=False, perf_mode=DR)
                nc.tensor.matmul(op[:], wbfr_ap[m][1], f8b3[:, :, px],
                                 start=False, stop=False, perf_mode=DR)
                nc.tensor.matmul(op[:], wbf_ap[m][0], fr8a3[:, :, px],
                                 start=False, stop=False, perf_mode=DR)
                nc.tensor.matmul(op[:], wbf_ap[m][1], fr8b3[:, :, px],
                                 start=False, stop=False, perf_mode=DR)

            def emit_attnT(t, attn8):
                aTp = p1k.tile([S_HALF, 1024], f8, tag="t1k", name="aTp",
                               padded_shape=[128, 1024])
                for u in range(4):
                    for h in range(2):
                        nc.tensor.transpose(
                            aTp[:, h * 512 + u * 128:h * 512 + (u + 1) * 128],
                            attn8[:, u * S_TOT + h * S_HALF:
                                  u * S_TOT + (h + 1) * S_HALF],
                            ident8[:])
                aT_sb = work_pool.tile([S_HALF, 1024], f8, tag="aTsb", name="aT_sb")
                copy_drain(aT_sb[:, :512], aTp[:, :512], 512)
                copy_drain(aT_sb[:, 512:], aTp[:, 512:], 512)
                return aT_sb

            def emit_ctx(t, m, op, aT_sb):
                px = bass.ts(t, PIX_T)
                aT3 = aT_sb[:].rearrange("s (i n) -> s i n", i=2)
                nc.tensor.matmul(op[:], upT_ap[m], aT3,
                                 start=False, stop=True, perf_mode=DR)
                o_sb = out_pool.tile([128, PIX_T], bf16, tag="osb", name="o_sb")
                relu_drain(o_sb[:], op[:], bco_ap[m], PIX_T)
                nc.sync.dma_start(out_d[m * 128:(m + 1) * 128, px], o_sb[:])

            for t in range(N_T):
                attn8 = emit_sim(t)
                ops = [pbig.tile([128, PIX_T], f32, tag="big", name="op")
                       for _ in range(4)]
                for m in range(3):
                    emit_bf(t, m, ops[m])
                aT_sb = emit_attnT(t, attn8)
                emit_bf(t, 3, ops[3])
                for m in range(4):
                    emit_ctx(t, m, ops[m], aT_sb)

    nc.compile()
    return nc


_NC_CACHE = {}
TRACE = False
LAST_RESULT = {}


def kernel(feats, w_kq, b_kq, w_v, b_v, w_out, b_out, w_bot, b_bot):
    from concourse.bass_utils import run_bass_kernel_spmd

    feats = np.asarray(feats, dtype=np.float32)
    w_kq = np.asarray(w_kq, dtype=np.float32)
    b_kq = np.asarray(b_kq, dtype=np.float32)
    w_v = np.asarray(w_v, dtype=np.float32)
    b_v = np.asarray(b_v, dtype=np.float32)
    w_out = np.asarray(w_out, dtype=np.float32)
    b_out = np.asarray(b_out, dtype=np.float32)
    w_bot = np.asarray(w_bot, dtype=np.float32)
    b_bot = np.asarray(b_bot, dtype=np.float32)

    # host-side weight prep
    w_co = w_bot[:, :C] @ w_out                     # [CO, CK]
    b_co = w_bot[:, :C] @ b_out + b_bot + w_co @ b_v
    w_bf = w_bot[:, C:]                             # [CO, C]

    # wq8/wv8: [k, mc*512 + p*256 + i*128 + m] = 64*w[mc*128+m, (2p+i)*128+k]
    def proj_pack(wm):
        arr = wm.T.reshape(2, 2, 128, 2, 128)       # [p, i, k, mc, m]
        return _f8(arr.transpose(2, 3, 0, 1, 4).reshape(128, 1024), S_W)

    wq8 = proj_pack(w_kq)
    wv8 = proj_pack(w_v)

    # wbf8: [k, p*1024 + i*512 + o] = 2048*w_bf[o, (2p+i)*128+k]
    arr = w_bf.T.reshape(2, 2, 128, CO)             # [p, i, k, o]
    wbf_lay = arr.transpose(2, 0, 1, 3).reshape(128, 2048)
    wbf8 = _f8(wbf_lay, S_BF)
    wbfr8 = _f8(wbf_lay - wbf8.astype(np.float32) / np.float32(S_BF), S_BF)

    # wco8: [k, i*512 + o] = 64*w_co[o, i*128+k]
    arr = w_co.T.reshape(2, 128, CO)                # [i, k, o]
    wco8 = _f8(arr.transpose(1, 0, 2).reshape(128, 1024), S_WCO)

    if "nc" not in _NC_CACHE:
        _NC_CACHE["nc"] = _build_kernel()
    nc = _NC_CACHE["nc"]

    sc = np.float32(1.0 / np.sqrt(CK))
    in_maps = []
    for core in range(N_CORES):
        b, half = core // 2, core % 2
        h0 = half * H_SH
        pmat, inv_area = _build_pool_mat(h0)
        biases = np.zeros((128, 8), dtype=np.float32)
        biases[:, 0:2] = (S_W * b_kq).reshape(2, 128).T
        biases[:, 2:6] = (S_BF * b_co).reshape(4, 128).T
        biases[:S_TOT, 6] = inv_area * sc * (S_KEY / S_Q)
        biases[:S_TOT, 7] = inv_area * (S_VP / S_Q)
        shard = np.ascontiguousarray(
            feats[b, :, h0:h0 + H_SH, :]).reshape(C, NPIX)
        f8v = _f8(shard).astype(np.float32)
        fr8v = _f8(shard - f8v)
        f8v = f8v.astype(F8)
        # pair layouts [128, 2*NPIX]: chunk-pair planes side by side
        c4 = f8v.reshape(4, 128, NPIX)
        r4 = fr8v.reshape(4, 128, NPIX)
        f8cat = np.concatenate(
            [np.concatenate([c4[0], c4[1]], axis=1),
             np.concatenate([c4[2], c4[3]], axis=1),
             np.concatenate([r4[0], r4[1]], axis=1),
             np.concatenate([r4[2], r4[3]], axis=1)], axis=1)
        in_maps.append(dict(
            f8cat=np.ascontiguousarray(f8cat), wq8=wq8, wv8=wv8,
            wbf8=wbf8, wbfr8=wbfr8, wco8=wco8,
            pmat=pmat.reshape(W, H_SH * S_TOT), biases=biases,
        ))

    res = run_bass_kernel_spmd(
        nc, in_maps, list(range(N_CORES)), trace=TRACE
    )
    LAST_RESULT["res"] = res

    out = np.empty((B, CO, H, W), dtype=np.float32)
    for core in range(N_CORES):
        b, half = core // 2, core % 2
        h0 = half * H_SH
        out[b, :, h0:h0 + H_SH, :] = (
            np.asarray(res.results[core]["out"]).astype(np.float32)
            * OUT_UNSCALE
        ).reshape(CO, H_SH, W)
    return out


# revision 12
# speedup vs baseline: 1.6169x; 1.0390x over previous
"""APNB (asymmetric pyramid non-local block) Trainium2 kernel, fp8 edition.

Full inputs in, full output out. Sharding: 8 cores = (batch b=core//2,
H-half=core%2). Each core handles feats[b, :, 64*half:64*half+64, :]
(= [512, 8192] pixels).

All heavy GEMMs run as fp8-e4m3 DoubleRow matmuls (2 k-chunks per
instruction). The precision-critical bottleneck conv on feats uses a
3-term fp8 residual decomposition (w8@f8 + w8@fr8 + wr8@f8, shared
power-of-2 scale 2048) which matches bf16 accuracy. The attention side
(q/v projections, PPM pooling, softmax, pooled-value contraction with
W_out folded into W_bot on the host) is plain fp8 with per-tensor
power-of-2 scales. Final output is produced at 2048x scale and
exponent-shifted back on the host (exact).

Engine budget: elementwise drains/copies are load-balanced across
Vector/Scalar/GpSimd with a greedy cost tracker; phases A/B are
interleaved and phase D is software-pipelined so the PE never starves.
"""

import sys

for p in ("/opt/trn_rl_repo",):
    if p not in sys.path:
        sys.path.insert(0, p)

import numpy as np
import ml_dtypes

BF16 = ml_dtypes.bfloat16
F8 = ml_dtypes.float8_e4m3  # TRN float8e4 (max +-240)

# ---- problem constants (hardcoded per spec) ----
B, C, H, W = 4, 512, 128, 128
CK, CO = 256, 512
POOL_SCALES = (1, 3, 6, 8)
S_TOT = sum(s * s for s in POOL_SCALES)  # 110
S_HALF = S_TOT // 2                      # 55
N_CORES = 8
H_SH = H // 2          # 64 rows per core
NPIX = H_SH * W        # 8192 pixels per core
PIX_T = 512            # gemm pixel tile
N_T = NPIX // PIX_T    # 16
N_PAIR = H_SH // 2     # 32 row pairs for DoubleRow pooling

# power-of-2 scales
S_W = 64.0        # w_kq / w_v
S_Q = 64.0        # q8 = 64*relu(q), v8 = 64*v
S_KEY = 32.0      # keys
S_VP = 32.0       # pooled v (channel-major)
S_WCO = 64.0      # folded W_co
S_UP = 32.0       # u_pool
S_ATTN = 64.0     # attn weights
S_BF = 2048.0     # bottleneck feats weight + out psum scale
EXP_SCALE = 1.0 / (S_Q * S_KEY)   # fold q/key scales out inside exp
U_DRAIN = S_UP / (S_WCO * S_VP)   # u psum -> u8
OUT_UNSCALE = np.float32(1.0 / S_BF)


def _pool_bounds(n, s):
    i = np.arange(s)
    return (i * n) // s, -((-(i + 1) * n) // s)


def _build_pool_mat(h0):
    """P[w, r, bin] = 1 if global pixel (h0+r, w) is in bin; fp8 (exact)."""
    P = np.zeros((W, H_SH, S_TOT), dtype=np.float32)
    inv_area = np.zeros((S_TOT,), dtype=np.float32)
    off = 0
    for s in POOL_SCALES:
        hs, he = _pool_bounds(H, s)
        ws, we = _pool_bounds(W, s)
        for i in range(s):
            for j in range(s):
                b = off + i * s + j
                inv_area[b] = 1.0 / float((he[i] - hs[i]) * (we[j] - ws[j]))
                r0 = max(hs[i] - h0, 0)
                r1 = min(he[i] - h0, H_SH)
                if r1 > r0:
                    P[ws[j]:we[j], r0:r1, b] = 1.0
        off += s * s
    return P.astype(F8), inv_area


def _f8(x, scale=1.0):
    y = np.asarray(x, np.float32) * np.float32(scale)
    y = np.clip(y, -240.0, 240.0)
    return y.astype(F8)


def _build_kernel(sim_mode=False):
    import concourse.bass as bass
    import concourse.bacc as bacc
    import concourse.mybir as mybir
    from concourse import tile, masks

    dt = mybir.dt
    f32, bf16, f8 = dt.float32, dt.bfloat16, dt.float8e4
    AF = mybir.ActivationFunctionType
    ALU = mybir.AluOpType
    DR = mybir.MatmulPerfMode.DoubleRow

    nc = bacc.Bacc("TRN2", target_bir_lowering=False, debug=False)

    # ---- DRAM I/O ----
    # f8cat: [f8a | f8b | fr8a | fr8b], each [128, 2*NPIX] chunk-pair layout
    f8cat_d = nc.dram_tensor("f8cat", [128, 8 * NPIX], f8, kind="ExternalInput")
    wq8_d = nc.dram_tensor("wq8", [128, 2 * 512], f8, kind="ExternalInput")
    wv8_d = nc.dram_tensor("wv8", [128, 2 * 512], f8, kind="ExternalInput")
    wbf8_d = nc.dram_tensor("wbf8", [128, 2048], f8, kind="ExternalInput")
    wbfr8_d = nc.dram_tensor("wbfr8", [128, 2048], f8, kind="ExternalInput")
    wco8_d = nc.dram_tensor("wco8", [128, 1024], f8, kind="ExternalInput")
    pmat_d = nc.dram_tensor("pmat", [W, H_SH * S_TOT], f8, kind="ExternalInput")
    bias_d = nc.dram_tensor("biases", [128, 8], f32, kind="ExternalInput")
    out_d = nc.dram_tensor("out", [CO, NPIX], bf16, kind="ExternalOutput")

    # greedy engine load balancer for drains/copies
    class Rot:
        def __init__(self):
            self.load = {"dve": 0.0, "act": 0.0, "pool": 0.0}

        def add(self, e, ns):
            self.load[e] += ns

        def pick(self, width, psum_in=True):
            cd = width * 1.05 + (130.0 if psum_in else 65.0)
            ca = width * 0.84 + 190.0
            cp = width * 1.39 + 125.0
            best = min(
                (self.load["dve"] + cd, cd, "dve"),
                (self.load["act"] + ca, ca, "act"),
                (self.load["pool"] + cp, cp, "pool"),
            )
            self.load[best[2]] = best[0]
            return best[2]

    rot = Rot()

    with tile.TileContext(nc) as tc:
        with (
            tc.tile_pool(name="const", bufs=1) as const_pool,
            tc.tile_pool(name="work", bufs=2) as work_pool,
            tc.tile_pool(name="outb", bufs=4) as out_pool,
            tc.tile_pool(name="pbig", bufs=4, space="PSUM") as pbig,
            tc.tile_pool(name="pacc", bufs=1, space="PSUM") as pacc,
            tc.tile_pool(name="p1k", bufs=2, space="PSUM") as p1k,
            tc.tile_pool(name="psim", bufs=1, space="PSUM") as psim,
            tc.tile_pool(name="dram", bufs=1, space="DRAM") as dram_pool,
        ):
            def relu_drain(out, in_, bias_ap, width):
                e = rot.pick(width)
                if e == "dve":
                    nc.vector.tensor_scalar(out, in_, bias_ap, 0.0, ALU.add, ALU.max)
                elif e == "act":
                    nc.scalar.activation(out, in_, AF.Relu, bias=bias_ap)
                else:
                    nc.gpsimd.tensor_scalar(out, in_, bias_ap, 0.0, ALU.add, ALU.max)

            def copy_drain(out, in_, width):
                e = rot.pick(width)
                if e == "dve":
                    nc.vector.tensor_copy(out, in_)
                elif e == "act":
                    nc.scalar.copy(out, in_)
                else:
                    nc.gpsimd.tensor_copy(out, in_)

            # ---- constants / weights ----
            ident8 = const_pool.tile([128, 128], f8, tag="ident", name="ident8")
            masks.make_identity(nc, ident8[:])

            # feats fp8 (pair layouts) + residuals
            f8a = const_pool.tile([128, 2 * NPIX], f8, tag="f8a", name="f8a")
            f8b = const_pool.tile([128, 2 * NPIX], f8, tag="f8b", name="f8b")
            fr8a = const_pool.tile([128, 2 * NPIX], f8, tag="fr8a", name="fr8a")
            fr8b = const_pool.tile([128, 2 * NPIX], f8, tag="fr8b", name="fr8b")
            f8a3 = f8a[:].rearrange("k (i n) -> k i n", i=2)
            f8b3 = f8b[:].rearrange("k (i n) -> k i n", i=2)
            fr8a3 = fr8a[:].rearrange("k (i n) -> k i n", i=2)
            fr8b3 = fr8b[:].rearrange("k (i n) -> k i n", i=2)

            wq8_sb = const_pool.tile([128, 1024], f8, tag="wq8", name="wq8_sb")
            wv8_sb = const_pool.tile([128, 1024], f8, tag="wv8", name="wv8_sb")
            bias_sb = const_pool.tile([128, 8], f32, tag="bias", name="bias_sb")
            pmat_sb = const_pool.tile([W, H_SH * S_TOT], f8, tag="pmat", name="pmat_sb")
            wbf8_sb = const_pool.tile([128, 2048], f8, tag="wbf8", name="wbf8_sb")
            wbfr8_sb = const_pool.tile([128, 2048], f8, tag="wbfr8", name="wbfr8_sb")
            wco8_sb = const_pool.tile([128, 1024], f8, tag="wco8", name="wco8_sb")

            def dma_f8_slices(t):
                g = t // 2
                c0 = g * 2 * PIX_T
                nc.sync.dma_start(f8a[:, c0:c0 + 1024], f8cat_d[:, c0:c0 + 1024])
                nc.sync.dma_start(
                    f8a[:, NPIX + c0:NPIX + c0 + 1024],
                    f8cat_d[:, NPIX + c0:NPIX + c0 + 1024])
                nc.sync.dma_start(
                    f8b[:, c0:c0 + 1024],
                    f8cat_d[:, 2 * NPIX + c0:2 * NPIX + c0 + 1024])
                nc.sync.dma_start(
                    f8b[:, NPIX + c0:NPIX + c0 + 1024],
                    f8cat_d[:, 3 * NPIX + c0:3 * NPIX + c0 + 1024])

            # first feats slices, then small consts
            dma_f8_slices(0)
            nc.sync.dma_start(wq8_sb[:], wq8_d[:])
            nc.sync.dma_start(wv8_sb[:], wv8_d[:])
            nc.sync.dma_start(bias_sb[:], bias_d[:])

            bkq_ap = [bias_sb[:, m:m + 1] for m in range(2)]            # 64*b_kq
            bco_ap = [bias_sb[:, 2 + m:3 + m] for m in range(4)]        # 2048*b_co
            iak_ap = bias_sb[:S_TOT, 6:7]                               # inv_area*sc/2
            iav_ap = bias_sb[:S_TOT, 7:8]                               # inv_area/2

            pmat3 = pmat_sb[:].rearrange("w (r s) -> w r s", s=S_TOT)

            # q8 = 64*relu(q), v8 = 64*v; [128, 2*NPIX]: m-chunk planes
            q8 = const_pool.tile([128, 2 * NPIX], f8, tag="q8", name="q8")
            v8 = const_pool.tile([128, 2 * NPIX], f8, tag="v8", name="v8")
            q83 = q8[:].rearrange("k (i n) -> k i n", i=2)

            # DR weight APs
            def pair_ap(tile_ap, base, width):
                return tile_ap[:, base:base + 2 * width].rearrange(
                    "k (i m) -> k i m", i=2)

            wq_ap = [[pair_ap(wq8_sb[:], mc * 512 + p * 256, 128)
                      for p in range(2)] for mc in range(2)]
            wv_ap = [[pair_ap(wv8_sb[:], mc * 512 + p * 256, 128)
                      for p in range(2)] for mc in range(2)]
            wbf_ap = [[pair_ap(wbf8_sb[:], p * 1024, 512)[:, :, m * 128:(m + 1) * 128]
                       for p in range(2)] for m in range(4)]
            wbfr_ap = [[pair_ap(wbfr8_sb[:], p * 1024, 512)[:, :, m * 128:(m + 1) * 128]
                        for p in range(2)] for m in range(4)]
            wco_ap = [pair_ap(wco8_sb[:], 0, 512)[:, :, o * 128:(o + 1) * 128]
                      for o in range(4)]

            # ---- phases A+B interleaved ----
            pooled_ps = pacc.tile([S_TOT, 512], f32, tag="pooled", name="pooled_ps")
            qvT_list = []

            def emit_a_tile(t):
                px = bass.ts(t, PIX_T)
                for mc in range(2):
                    qp = pbig.tile([128, PIX_T], f32, tag="big", name="qp")
                    nc.tensor.matmul(qp[:], wq_ap[mc][0], f8a3[:, :, px],
                                     start=True, stop=False, perf_mode=DR)
                    nc.tensor.matmul(qp[:], wq_ap[mc][1], f8b3[:, :, px],
                                     start=False, stop=True, perf_mode=DR)
                    relu_drain(q8[:, mc * NPIX + t * PIX_T:
                                  mc * NPIX + (t + 1) * PIX_T],
                               qp[:], bkq_ap[mc], PIX_T)
                for mc in range(2):
                    vp = pbig.tile([128, PIX_T], f32, tag="big", name="vp")
                    nc.tensor.matmul(vp[:], wv_ap[mc][0], f8a3[:, :, px],
                                     start=True, stop=False, perf_mode=DR)
                    nc.tensor.matmul(vp[:], wv_ap[mc][1], f8b3[:, :, px],
                                     start=False, stop=True, perf_mode=DR)
                    copy_drain(v8[:, mc * NPIX + t * PIX_T:
                                  mc * NPIX + (t + 1) * PIX_T],
                               vp[:], PIX_T)

            def emit_b_pair(p):
                rowT = p1k.tile([128, 1024], f8, tag="t1k", name="rowT")
                for j in range(2):
                    r = 2 * p + j
                    for mc in range(2):
                        nc.tensor.transpose(
                            rowT[:, j * 512 + mc * 128:j * 512 + (mc + 1) * 128],
                            q8[:, mc * NPIX + r * 128:mc * NPIX + (r + 1) * 128],
                            ident8[:])
                        nc.tensor.transpose(
                            rowT[:, j * 512 + 256 + mc * 128:j * 512 + 256 + (mc + 1) * 128],
                            v8[:, mc * NPIX + r * 128:mc * NPIX + (r + 1) * 128],
                            ident8[:])
                qvT = work_pool.tile([128, 1024], f8, tag="qvT", name="qvT", bufs=3)
                copy_drain(qvT[:], rowT[:], 1024)
                qvT_list.append(qvT)

            def emit_pool_mm(pp):
                nc.tensor.matmul(
                    pooled_ps[:], pmat3[:, 2 * pp:2 * pp + 2, :],
                    qvT_list[pp][:].rearrange("w (i c) -> w i c", i=2),
                    start=(pp == 0), stop=(pp == N_PAIR - 1), perf_mode=DR)

            for t in range(N_T):
                if t >= 2 and t % 2 == 0:
                    dma_f8_slices(t)
                if 1 <= t <= 4:   # pmat in 4 row-chunks (16 rows each)
                    g = t - 1
                    nc.sync.dma_start(
                        pmat_sb[:, g * 16 * S_TOT:(g + 1) * 16 * S_TOT],
                        pmat_d[:, g * 16 * S_TOT:(g + 1) * 16 * S_TOT])
                if 5 <= t <= 12:  # fr8 in 8 column-chunks
                    g = t - 5
                    base = 4 * NPIX + g * 2048
                    dst = fr8a if g < 4 else fr8b
                    off = (g % 4) * 2048
                    nc.sync.dma_start(dst[:, off:off + 2048],
                                      f8cat_d[:, base:base + 2048])
                if t == 13:
                    nc.sync.dma_start(wbf8_sb[:], wbf8_d[:])
                if t == 14:
                    nc.sync.dma_start(wbfr8_sb[:], wbfr8_d[:])
                if t == 15:
                    nc.sync.dma_start(wco8_sb[:], wco8_d[:])
                emit_a_tile(t)
                if t >= 2:
                    for p in (2 * (t - 2), 2 * (t - 2) + 1):
                        emit_b_pair(p)
                        if p >= 2:
                            emit_pool_mm(p - 2)
            for p in range(2 * (N_T - 2), N_PAIR):
                emit_b_pair(p)
                if p >= 2:
                    emit_pool_mm(p - 2)
            for pp in range(N_PAIR - 2, N_PAIR):
                emit_pool_mm(pp)

            # ---- phase C: AllReduce + pooled-side prep ----
            pooled_sb = work_pool.tile([S_TOT, 512], f32, tag="pooled", name="pooled_sb", bufs=1)
            nc.vector.tensor_copy(pooled_sb[:], pooled_ps[:])
            rot.add("dve", 700)
            cc_in = dram_pool.tile([S_TOT, 512], f32, tag="cc_in", name="cc_in")
            cc_out = dram_pool.tile([S_TOT, 512], f32, tag="cc_out", name="cc_out")
            nc.sync.dma_start(cc_in[:], pooled_sb[:])
            if sim_mode:
                nc.sync.dma_start(cc_out[:], cc_in[:])
            else:
                nc.gpsimd.collective_compute(
                    "AllReduce",
                    ALU.add,
                    replica_groups=[[0, 1], [2, 3], [4, 5], [6, 7]],
                    ins=[cc_in.opt()],
                    outs=[cc_out.opt()],
                )
            pooled_f = work_pool.tile([S_TOT, 512], f32, tag="pooled", name="pooled_f", bufs=1)
            nc.sync.dma_start(pooled_f[:], cc_out[:])

            # tile 0's bottleneck-conv matmuls: independent of the collective,
            # keep the PE busy during the AllReduce round trip
            def emit_bf(t, m, op):
                px = bass.ts(t, PIX_T)
                nc.tensor.matmul(op[:], wbf_ap[m][0], f8a3[:, :, px],
                                 start=True, stop=False, perf_mode=DR)
                nc.tensor.matmul(op[:], wbf_ap[m][1], f8b3[:, :, px],
                                 start=False, stop=False, perf_mode=DR)
                nc.tensor.matmul(op[:], wbfr_ap[m][0], f8a3[:, :, px],
                                 start=False, stop=False, perf_mode=DR)
                nc.tensor.matmul(op[:], wbfr_ap[m][1], f8b3[:, :, px],
                                 start=False, stop=False, perf_mode=DR)
                nc.tensor.matmul(op[:], wbf_ap[m][0], fr8a3[:, :, px],
                                 start=False, stop=False, perf_mode=DR)
                nc.tensor.matmul(op[:], wbf_ap[m][1], fr8b3[:, :, px],
                                 start=False, stop=False, perf_mode=DR)

            # out psums rotate over 5 banks: the retired pooled bank + pbig's 4
            def alloc_ops():
                return [pacc.tile([128, PIX_T], f32, tag="pooled", name="op",
                                  padded_shape=[128, 512])] + \
                       [pbig.tile([128, PIX_T], f32, tag="big", name="op")
                        for _ in range(3)]

            ops0 = alloc_ops()
            for m in range(4):
                emit_bf(0, m, ops0[m])

            # keyval8: [110, 512] fp8 = [32*keys*sc | 32*v_pool]
            keyval8 = const_pool.tile([S_TOT, 512], f8, tag="keyval", name="keyval8")
            nc.vector.tensor_scalar(
                keyval8[:, :CK], pooled_f[:, :CK], iak_ap, None, ALU.mult)
            nc.vector.tensor_scalar(
                keyval8[:, CK:], pooled_f[:, CK:], iav_ap, None, ALU.mult)
            rot.add("dve", 700)

            # key_cm / v_cm channel-major pair layouts [128, 2*110]
            kt = p1k.tile([128, 2 * S_TOT], f8, tag="t1k", name="kt",
                          padded_shape=[128, 1024])
            for mc in range(2):
                nc.tensor.transpose(
                    kt[:, mc * S_TOT:(mc + 1) * S_TOT],
                    keyval8[:, mc * 128:(mc + 1) * 128],
                    ident8[:S_TOT, :S_TOT])
            key_cm = const_pool.tile([128, 2 * S_TOT], f8, tag="keycm", name="key_cm")
            nc.scalar.copy(key_cm[:], kt[:])
            rot.add("act", 400)
            key3 = key_cm[:].rearrange("k (i s) -> k i s", i=2)

            vt = p1k.tile([128, 2 * S_TOT], f8, tag="t1k", name="vt",
                          padded_shape=[128, 1024])
            for mc in range(2):
                nc.tensor.transpose(
                    vt[:, mc * S_TOT:(mc + 1) * S_TOT],
                    keyval8[:, CK + mc * 128:CK + (mc + 1) * 128],
                    ident8[:S_TOT, :S_TOT])
            v_cm = const_pool.tile([128, 2 * S_TOT], f8, tag="vcm", name="v_cm")
            nc.scalar.copy(v_cm[:], vt[:])
            rot.add("act", 400)
            v3 = v_cm[:].rearrange("k (i s) -> k i s", i=2)

            # u_pool = W_co @ v_pool (DR), drained to fp8 at 32x
            u_ps = psim.tile([128, 4 * S_TOT], f32, tag="sim", name="u_ps",
                             padded_shape=[128, 512])
            for o in range(4):
                nc.tensor.matmul(u_ps[:, o * S_TOT:(o + 1) * S_TOT],
                                 wco_ap[o], v3, start=True, stop=True,
                                 perf_mode=DR)
            u_sb = const_pool.tile([128, 4 * S_TOT], f8, tag="usb", name="u_sb")
            nc.vector.tensor_scalar(u_sb[:], u_ps[:], U_DRAIN, None, ALU.mult)
            rot.add("dve", 600)

            # u_poolT in [55, 2, 128] DR layout per o-chunk
            upT_ps = p1k.tile([S_HALF, 4 * 256], f8, tag="t1k", name="upT_ps",
                              padded_shape=[128, 1024])
            for o in range(4):
                for h in range(2):
                    nc.tensor.transpose(
                        upT_ps[:, o * 256 + h * 128:o * 256 + (h + 1) * 128],
                        u_sb[:, o * S_TOT + h * S_HALF:o * S_TOT + (h + 1) * S_HALF],
                        ident8[:])
            upT_sb = const_pool.tile([S_HALF, 4 * 256], f8, tag="upT", name="upT_sb")
            nc.scalar.copy(upT_sb[:], upT_ps[:])
            rot.add("act", 1000)
            upT_ap = [upT_sb[:, o * 256:(o + 1) * 256].rearrange(
                "s (i m) -> s i m", i=2) for o in range(4)]

            # ---- phase D: attention + fused output (software pipelined) ----
            def emit_sim(t):
                sim = psim.tile([128, 4 * S_TOT], f32, tag="sim", name="sim",
                                padded_shape=[128, 512])
                for u in range(4):
                    upx = bass.ts(t * 4 + u, 128)
                    nc.tensor.matmul(
                        sim[:, u * S_TOT:(u + 1) * S_TOT],
                        q83[:, :, upx], key3, start=True, stop=True,
                        perf_mode=DR)
                attn_f = work_pool.tile([128, 4 * S_TOT], f32, tag="attnf", name="attn_f")
                nc.scalar.activation(attn_f[:], sim[:], AF.Exp, scale=EXP_SCALE)
                rot.add("act", 750)
                den = work_pool.tile([128, 4], f32, tag="den", name="den")
                nc.vector.tensor_reduce(
                    den[:], attn_f[:].rearrange("p (u s) -> p u s", s=S_TOT),
                    axis=mybir.AxisListType.X, op=ALU.add)
                rden = work_pool.tile([128, 4], f32, tag="rden", name="rden")
                nc.vector.reciprocal(rden[:], den[:])
                attn8 = work_pool.tile([128, 4 * S_TOT], f8, tag="attn8", name="attn8")
                for u in range(4):
                    usl = bass.ts(u, S_TOT)
                    nc.vector.tensor_scalar(
                        attn8[:, usl], attn_f[:, usl], rden[:, u:u + 1],
                        S_ATTN, ALU.mult, ALU.mult)
                rot.add("dve", 2000)
                return attn8

            def emit_attnT(t, attn8):
                aTp = p1k.tile([S_HALF, 1024], f8, tag="t1k", name="aTp",
                               padded_shape=[128, 1024])
                for u in range(4):
                    for h in range(2):
                        nc.tensor.transpose(
                            aTp[:, h * 512 + u * 128:h * 512 + (u + 1) * 128],
                            attn8[:, u * S_TOT + h * S_HALF:
                                  u * S_TOT + (h + 1) * S_HALF],
                            ident8[:])
                aT_sb = work_pool.tile([S_HALF, 1024], f8, tag="aTsb", name="aT_sb")
                copy_drain(aT_sb[:, :512], aTp[:, :512], 512)
                copy_drain(aT_sb[:, 512:], aTp[:, 512:], 512)
                return aT_sb

            def emit_ctx(t, m, op, aT_sb):
                px = bass.ts(t, PIX_T)
                aT3 = aT_sb[:].rearrange("s (i n) -> s i n", i=2)
                nc.tensor.matmul(op[:], upT_ap[m], aT3,
                                 start=False, stop=True, perf_mode=DR)
                o_sb = out_pool.tile([128, PIX_T], bf16, tag="osb", name="o_sb")
                relu_drain(o_sb[:], op[:], bco_ap[m], PIX_T)
                nc.sync.dma_start(out_d[m * 128:(m + 1) * 128, px], o_sb[:])

            attn8_cur = emit_sim(0)
            for t in range(N_T):
                attn8_next = emit_sim(t + 1) if t + 1 < N_T else None
                aT_sb = emit_attnT(t, attn8_cur)
                if t == 0:
                    ops = ops0
                else:
                    ops = alloc_ops()
                    for m in range(4):
                        emit_bf(t, m, ops[m])
                for m in range(4):
                    emit_ctx(t, m, ops[m], aT_sb)
                attn8_cur = attn8_next

    nc.compile()
    return nc


_NC_CACHE = {}
TRACE = False
LAST_RESULT = {}


def kernel(feats, w_kq, b_kq, w_v, b_v, w_out, b_out, w_bot, b_bot):
    from concourse.bass_utils import run_bass_kernel_spmd

    feats = np.asarray(feats, dtype=np.float32)
    w_kq = np.asarray(w_kq, dtype=np.float32)
    b_kq = np.asarray(b_kq, dtype=np.float32)
    w_v = np.asarray(w_v, dtype=np.float32)
    b_v = np.asarray(b_v, dtype=np.float32)
    w_out = np.asarray(w_out, dtype=np.float32)
    b_out = np.asarray(b_out, dtype=np.float32)
    w_bot = np.asarray(w_bot, dtype=np.float32)
    b_bot = np.asarray(b_bot, dtype=np.float32)

    # host-side weight prep
    w_co = w_bot[:, :C] @ w_out                     # [CO, CK]
    b_co = w_bot[:, :C] @ b_out + b_bot + w_co @ b_v
    w_bf = w_bot[:, C:]                             # [CO, C]

    # wq8/wv8: [k, mc*512 + p*256 + i*128 + m] = 64*w[mc*128+m, (2p+i)*128+k]
    def proj_pack(wm):
        arr = wm.T.reshape(2, 2, 128, 2, 128)       # [p, i, k, mc, m]
        return _f8(arr.transpose(2, 3, 0, 1, 4).reshape(128, 1024), S_W)

    wq8 = proj_pack(w_kq)
    wv8 = proj_pack(w_v)

    # wbf8: [k, p*1024 + i*512 + o] = 2048*w_bf[o, (2p+i)*128+k]
    arr = w_bf.T.reshape(2, 2, 128, CO)             # [p, i, k, o]
    wbf_lay = arr.transpose(2, 0, 1, 3).reshape(128, 2048)
    wbf8 = _f8(wbf_lay, S_BF)
    wbfr8 = _f8(wbf_lay - wbf8.astype(np.float32) / np.float32(S_BF), S_BF)

    # wco8: [k, i*512 + o] = 64*w_co[o, i*128+k]
    arr = w_co.T.reshape(2, 128, CO)                # [i, k, o]
    wco8 = _f8(arr.transpose(1, 0, 2).reshape(128, 1024), S_WCO)

    if "nc" not in _NC_CACHE:
        _NC_CACHE["nc"] = _build_kernel()
    nc = _NC_CACHE["nc"]

    sc = np.float32(1.0 / np.sqrt(CK))
    in_maps = []
    for core in range(N_CORES):
        b, half = core // 2, core % 2
        h0 = half * H_SH
        pmat, inv_area = _build_pool_mat(h0)
        biases = np.zeros((128, 8), dtype=np.float32)
        biases[:, 0:2] = (S_W * b_kq).reshape(2, 128).T
        biases[:, 2:6] = (S_BF * b_co).reshape(4, 128).T
        biases[:S_TOT, 6] = inv_area * sc * (S_KEY / S_Q)
        biases[:S_TOT, 7] = inv_area * (S_VP / S_Q)
        shard = np.ascontiguousarray(
            feats[b, :, h0:h0 + H_SH, :]).reshape(C, NPIX)
        f8v = _f8(shard).astype(np.float32)
        fr8v = _f8(shard - f8v)
        f8v = f8v.astype(F8)
        # pair layouts [128, 2*NPIX]: chunk-pair planes side by side
        c4 = f8v.reshape(4, 128, NPIX)
        r4 = fr8v.reshape(4, 128, NPIX)
        f8cat = np.concatenate(
            [np.concatenate([c4[0], c4[1]], axis=1),
             np.concatenate([c4[2], c4[3]], axis=1),
             np.concatenate([r4[0], r4[1]], axis=1),
             np.concatenate([r4[2], r4[3]], axis=1)], axis=1)
        in_maps.append(dict(
            f8cat=np.ascontiguousarray(f8cat), wq8=wq8, wv8=wv8,
            wbf8=wbf8, wbfr8=wbfr8, wco8=wco8,
            pmat=pmat.reshape(W, H_SH * S_TOT), biases=biases,
        ))

    res = run_bass_kernel_spmd(
        nc, in_maps, list(range(N_CORES)), trace=TRACE
    )
    LAST_RESULT["res"] = res

    out = np.empty((B, CO, H, W), dtype=np.float32)
    for core in range(N_CORES):
        b, half = core // 2, core % 2
        h0 = half * H_SH
        out[b, :, h0:h0 + H_SH, :] = (
            np.asarray(res.results[core]["out"]).astype(np.float32)
            * OUT_UNSCALE
        ).reshape(CO, H_SH, W)
    return out


# revision 18
# speedup vs baseline: 1.6715x; 1.0338x over previous
"""APNB (asymmetric pyramid non-local block) Trainium2 kernel, fp8 edition.

Full inputs in, full output out. Sharding: 8 cores = (batch b=core//2,
H-half=core%2). Each core handles feats[b, :, 64*half:64*half+64, :]
(= [512, 8192] pixels).

All heavy GEMMs run as fp8-e4m3 DoubleRow matmuls (2 k-chunks per
instruction). The precision-critical bottleneck conv on feats uses a
3-term fp8 residual decomposition (w8@f8 + w8@fr8 + wr8@f8, shared
power-of-2 scale 2048) which matches bf16 accuracy. The attention side
(q/v projections, PPM pooling, softmax, pooled-value contraction with
W_out folded into W_bot on the host) is plain fp8 with per-tensor
power-of-2 scales. Final output is produced at 2048x scale and
exponent-shifted back on the host (exact).

Engine budget: elementwise drains/copies are load-balanced across
Vector/Scalar/GpSimd with a greedy cost tracker; phases A/B are
interleaved and phase D is software-pipelined so the PE never starves.
"""

import sys

for p in ("/opt/trn_rl_repo",):
    if p not in sys.path:
        sys.path.insert(0, p)

import numpy as np
import ml_dtypes

BF16 = ml_dtypes.bfloat16
F8 = ml_dtypes.float8_e4m3  # TRN float8e4 (max +-240)

# ---- problem constants (hardcoded per spec) ----
B, C, H, W = 4, 512, 128, 128
CK, CO = 256, 512
POOL_SCALES = (1, 3, 6, 8)
S_TOT = sum(s * s for s in POOL_SCALES)  # 110
S_HALF = S_TOT // 2                      # 55
N_CORES = 8
H_SH = H // 2          # 64 rows per core
NPIX = H_SH * W        # 8192 pixels per core
PIX_T = 512            # gemm pixel tile
N_T = NPIX // PIX_T    # 16
N_PAIR = H_SH // 2     # 32 row pairs for DoubleRow pooling

# power-of-2 scales
S_W = 64.0        # w_kq / w_v
S_Q = 64.0        # q8 = 64*relu(q), v8 = 64*v
S_KEY = 32.0      # keys
S_VP = 32.0       # pooled v (channel-major)
S_WCO = 64.0      # folded W_co
S_UP = 32.0       # u_pool
S_ATTN = 64.0     # attn weights
S_BF = 2048.0     # bottleneck feats weight + out psum scale
EXP_SCALE = 1.0 / (S_Q * S_KEY)   # fold q/key scales out inside exp
U_DRAIN = S_UP / (S_WCO * S_VP)   # u psum -> u8
OUT_UNSCALE = np.float32(1.0 / S_BF)


def _pool_bounds(n, s):
    i = np.arange(s)
    return (i * n) // s, -((-(i + 1) * n) // s)


def _build_pool_mat(h0):
    """P[w, r, bin] = 1 if global pixel (h0+r, w) is in bin; fp8 (exact)."""
    P = np.zeros((W, H_SH, S_TOT), dtype=np.float32)
    inv_area = np.zeros((S_TOT,), dtype=np.float32)
    off = 0
    for s in POOL_SCALES:
        hs, he = _pool_bounds(H, s)
        ws, we = _pool_bounds(W, s)
        for i in range(s):
            for j in range(s):
                b = off + i * s + j
                inv_area[b] = 1.0 / float((he[i] - hs[i]) * (we[j] - ws[j]))
                r0 = max(hs[i] - h0, 0)
                r1 = min(he[i] - h0, H_SH)
                if r1 > r0:
                    P[ws[j]:we[j], r0:r1, b] = 1.0
        off += s * s
    return P.astype(F8), inv_area


def _f8(x, scale=1.0):
    y = np.asarray(x, np.float32) * np.float32(scale)
    y = np.clip(y, -240.0, 240.0)
    return y.astype(F8)


def _build_kernel(sim_mode=False):
    import concourse.bass as bass
    import concourse.bacc as bacc
    import concourse.mybir as mybir
    from concourse import tile, masks

    dt = mybir.dt
    f32, bf16, f8 = dt.float32, dt.bfloat16, dt.float8e4
    AF = mybir.ActivationFunctionType
    ALU = mybir.AluOpType
    DR = mybir.MatmulPerfMode.DoubleRow

    nc = bacc.Bacc("TRN2", target_bir_lowering=False, debug=False)

    # ---- DRAM I/O ----
    # f8cat: [f8a | f8b | fr8a | fr8b], each [128, 2*NPIX] chunk-pair layout
    f8cat_d = nc.dram_tensor("f8cat", [128, 8 * NPIX], f8, kind="ExternalInput")
    wq8_d = nc.dram_tensor("wq8", [128, 2 * 512], f8, kind="ExternalInput")
    wv8_d = nc.dram_tensor("wv8", [128, 2 * 512], f8, kind="ExternalInput")
    wbf8_d = nc.dram_tensor("wbf8", [128, 2048], f8, kind="ExternalInput")
    wbfr8_d = nc.dram_tensor("wbfr8", [128, 2048], f8, kind="ExternalInput")
    wco8_d = nc.dram_tensor("wco8", [128, 1024], f8, kind="ExternalInput")
    pmat_d = nc.dram_tensor("pmat", [W, H_SH * S_TOT], f8, kind="ExternalInput")
    bias_d = nc.dram_tensor("biases", [128, 8], f32, kind="ExternalInput")
    out_d = nc.dram_tensor("out", [CO, NPIX], bf16, kind="ExternalOutput")

    # greedy engine load balancer for drains/copies
    class Rot:
        def __init__(self):
            self.load = {"dve": 0.0, "act": 0.0, "pool": 0.0}

        def add(self, e, ns):
            self.load[e] += ns

        def pick(self, width, psum_in=True):
            return self.pick2(width, ("dve", "act", "pool"), psum_in)

        def pick2(self, width, engines, psum_in=True):
            cost = {
                "dve": width * 1.05 + (130.0 if psum_in else 65.0),
                "act": width * 0.84 + 190.0,
                "pool": width * 1.39 + 125.0,
            }
            best = min((self.load[e] + cost[e], cost[e], e) for e in engines)
            self.load[best[2]] = best[0]
            return best[2]

    rot = Rot()

    with tile.TileContext(nc) as tc:
        with (
            tc.tile_pool(name="const", bufs=1) as const_pool,
            tc.tile_pool(name="work", bufs=2) as work_pool,
            tc.tile_pool(name="outb", bufs=4) as out_pool,
            tc.tile_pool(name="pbig", bufs=4, space="PSUM") as pbig,
            tc.tile_pool(name="pacc", bufs=1, space="PSUM") as pacc,
            tc.tile_pool(name="p1k", bufs=2, space="PSUM") as p1k,
            tc.tile_pool(name="psim", bufs=1, space="PSUM") as psim,
            tc.tile_pool(name="dram", bufs=1, space="DRAM") as dram_pool,
        ):
            def relu_drain(out, in_, bias_ap, width, engines=None):
                e = rot.pick(width) if engines is None else rot.pick2(width, engines)
                if e == "dve":
                    nc.vector.tensor_scalar(out, in_, bias_ap, 0.0, ALU.add, ALU.max)
                elif e == "act":
                    nc.scalar.activation(out, in_, AF.Relu, bias=bias_ap)
                else:
                    nc.gpsimd.tensor_scalar(out, in_, bias_ap, 0.0, ALU.add, ALU.max)

            def copy_drain(out, in_, width):
                e = rot.pick(width)
                if e == "dve":
                    nc.vector.tensor_copy(out, in_)
                elif e == "act":
                    nc.scalar.copy(out, in_)
                else:
                    nc.gpsimd.tensor_copy(out, in_)

            # ---- constants / weights ----
            ident8 = const_pool.tile([128, 128], f8, tag="ident", name="ident8")
            masks.make_identity(nc, ident8[:])

            # feats fp8 (pair layouts) + residuals
            f8a = const_pool.tile([128, 2 * NPIX], f8, tag="f8a", name="f8a")
            f8b = const_pool.tile([128, 2 * NPIX], f8, tag="f8b", name="f8b")
            fr8a = const_pool.tile([128, 2 * NPIX], f8, tag="fr8a", name="fr8a")
            fr8b = const_pool.tile([128, 2 * NPIX], f8, tag="fr8b", name="fr8b")
            f8a3 = f8a[:].rearrange("k (i n) -> k i n", i=2)
            f8b3 = f8b[:].rearrange("k (i n) -> k i n", i=2)
            fr8a3 = fr8a[:].rearrange("k (i n) -> k i n", i=2)
            fr8b3 = fr8b[:].rearrange("k (i n) -> k i n", i=2)

            wq8_sb = const_pool.tile([128, 1024], f8, tag="wq8", name="wq8_sb")
            wv8_sb = const_pool.tile([128, 1024], f8, tag="wv8", name="wv8_sb")
            bias_sb = const_pool.tile([128, 8], f32, tag="bias", name="bias_sb")
            pmat_sb = const_pool.tile([W, H_SH * S_TOT], f8, tag="pmat", name="pmat_sb")
            wbf8_sb = const_pool.tile([128, 2048], f8, tag="wbf8", name="wbf8_sb")
            wbfr8_sb = const_pool.tile([128, 2048], f8, tag="wbfr8", name="wbfr8_sb")
            wco8_sb = const_pool.tile([128, 1024], f8, tag="wco8", name="wco8_sb")

            def dma_f8_slices(t):
                g = t // 2
                c0 = g * 2 * PIX_T
                nc.sync.dma_start(f8a[:, c0:c0 + 1024], f8cat_d[:, c0:c0 + 1024])
                nc.sync.dma_start(
                    f8a[:, NPIX + c0:NPIX + c0 + 1024],
                    f8cat_d[:, NPIX + c0:NPIX + c0 + 1024])
                nc.sync.dma_start(
                    f8b[:, c0:c0 + 1024],
                    f8cat_d[:, 2 * NPIX + c0:2 * NPIX + c0 + 1024])
                nc.sync.dma_start(
                    f8b[:, NPIX + c0:NPIX + c0 + 1024],
                    f8cat_d[:, 3 * NPIX + c0:3 * NPIX + c0 + 1024])

            # first feats slices, then small consts
            dma_f8_slices(0)
            nc.sync.dma_start(wq8_sb[:], wq8_d[:])
            nc.sync.dma_start(wv8_sb[:], wv8_d[:])
            nc.sync.dma_start(bias_sb[:], bias_d[:])

            bkq_ap = [bias_sb[:, m:m + 1] for m in range(2)]            # 64*b_kq
            bco_ap = [bias_sb[:, 2 + m:3 + m] for m in range(4)]        # 2048*b_co
            iak_ap = bias_sb[:S_TOT, 6:7]                               # inv_area*sc/2
            iav_ap = bias_sb[:S_TOT, 7:8]                               # inv_area/2

            pmat3 = pmat_sb[:].rearrange("w (r s) -> w r s", s=S_TOT)

            # q8 = 64*relu(q), v8 = 64*v; [128, 2*NPIX]: m-chunk planes
            q8 = const_pool.tile([128, 2 * NPIX], f8, tag="q8", name="q8")
            v8 = const_pool.tile([128, 2 * NPIX], f8, tag="v8", name="v8")
            q83 = q8[:].rearrange("k (i n) -> k i n", i=2)

            # DR weight APs
            def pair_ap(tile_ap, base, width):
                return tile_ap[:, base:base + 2 * width].rearrange(
                    "k (i m) -> k i m", i=2)

            wq_ap = [[pair_ap(wq8_sb[:], mc * 512 + p * 256, 128)
                      for p in range(2)] for mc in range(2)]
            wv_ap = [[pair_ap(wv8_sb[:], mc * 512 + p * 256, 128)
                      for p in range(2)] for mc in range(2)]
            wbf_ap = [[pair_ap(wbf8_sb[:], p * 1024, 512)[:, :, m * 128:(m + 1) * 128]
                       for p in range(2)] for m in range(4)]
            wbfr_ap = [[pair_ap(wbfr8_sb[:], p * 1024, 512)[:, :, m * 128:(m + 1) * 128]
                        for p in range(2)] for m in range(4)]
            wco_ap = [pair_ap(wco8_sb[:], 0, 512)[:, :, o * 128:(o + 1) * 128]
                      for o in range(4)]

            # ---- phases A+B interleaved ----
            pooled_ps = pacc.tile([S_TOT, 512], f32, tag="pooled", name="pooled_ps")
            qvT_list = []

            def emit_a_tile(t):
                px = bass.ts(t, PIX_T)
                for mc in range(2):
                    qp = pbig.tile([128, PIX_T], f32, tag="big", name="qp")
                    nc.tensor.matmul(qp[:], wq_ap[mc][0], f8a3[:, :, px],
                                     start=True, stop=False, perf_mode=DR)
                    nc.tensor.matmul(qp[:], wq_ap[mc][1], f8b3[:, :, px],
                                     start=False, stop=True, perf_mode=DR)
                    relu_drain(q8[:, mc * NPIX + t * PIX_T:
                                  mc * NPIX + (t + 1) * PIX_T],
                               qp[:], bkq_ap[mc], PIX_T)
                for mc in range(2):
                    vp = pbig.tile([128, PIX_T], f32, tag="big", name="vp")
                    nc.tensor.matmul(vp[:], wv_ap[mc][0], f8a3[:, :, px],
                                     start=True, stop=False, perf_mode=DR)
                    nc.tensor.matmul(vp[:], wv_ap[mc][1], f8b3[:, :, px],
                                     start=False, stop=True, perf_mode=DR)
                    copy_drain(v8[:, mc * NPIX + t * PIX_T:
                                  mc * NPIX + (t + 1) * PIX_T],
                               vp[:], PIX_T)

            def emit_b_pair(p):
                rowT = p1k.tile([128, 1024], f8, tag="t1k", name="rowT")
                for j in range(2):
                    r = 2 * p + j
                    for mc in range(2):
                        nc.tensor.transpose(
                            rowT[:, j * 512 + mc * 128:j * 512 + (mc + 1) * 128],
                            q8[:, mc * NPIX + r * 128:mc * NPIX + (r + 1) * 128],
                            ident8[:])
                        nc.tensor.transpose(
                            rowT[:, j * 512 + 256 + mc * 128:j * 512 + 256 + (mc + 1) * 128],
                            v8[:, mc * NPIX + r * 128:mc * NPIX + (r + 1) * 128],
                            ident8[:])
                qvT = work_pool.tile([128, 1024], f8, tag="qvT", name="qvT", bufs=3)
                copy_drain(qvT[:], rowT[:], 1024)
                qvT_list.append(qvT)

            def emit_pool_mm(pp):
                nc.tensor.matmul(
                    pooled_ps[:], pmat3[:, 2 * pp:2 * pp + 2, :],
                    qvT_list[pp][:].rearrange("w (i c) -> w i c", i=2),
                    start=(pp == 0), stop=(pp == N_PAIR - 1), perf_mode=DR)

            for t in range(N_T):
                if t >= 2 and t % 2 == 0:
                    dma_f8_slices(t)
                if 1 <= t <= 4:   # pmat in 4 row-chunks (16 rows each)
                    g = t - 1
                    nc.sync.dma_start(
                        pmat_sb[:, g * 16 * S_TOT:(g + 1) * 16 * S_TOT],
                        pmat_d[:, g * 16 * S_TOT:(g + 1) * 16 * S_TOT])
                if 5 <= t <= 12:  # fr8 in 8 column-chunks
                    g = t - 5
                    base = 4 * NPIX + g * 2048
                    dst = fr8a if g < 4 else fr8b
                    off = (g % 4) * 2048
                    nc.sync.dma_start(dst[:, off:off + 2048],
                                      f8cat_d[:, base:base + 2048])
                if t == 13:
                    nc.sync.dma_start(wbf8_sb[:], wbf8_d[:])
                if t == 14:
                    nc.sync.dma_start(wbfr8_sb[:], wbfr8_d[:])
                if t == 15:
                    nc.sync.dma_start(wco8_sb[:], wco8_d[:])
                emit_a_tile(t)
                if t >= 2:
                    for p in (2 * (t - 2), 2 * (t - 2) + 1):
                        emit_b_pair(p)
                        if p >= 2:
                            emit_pool_mm(p - 2)
            for p in range(2 * (N_T - 2), N_PAIR):
                emit_b_pair(p)
                if p >= 2:
                    emit_pool_mm(p - 2)
            for pp in range(N_PAIR - 2, N_PAIR):
                emit_pool_mm(pp)

            # ---- phase C: AllReduce + pooled-side prep ----
            pooled_sb = work_pool.tile([S_TOT, 512], f32, tag="pooled", name="pooled_sb", bufs=1)
            nc.vector.tensor_copy(pooled_sb[:], pooled_ps[:])
            rot.add("dve", 700)
            cc_in = dram_pool.tile([S_TOT, 512], f32, tag="cc_in", name="cc_in")
            cc_out = dram_pool.tile([S_TOT, 512], f32, tag="cc_out", name="cc_out")
            nc.sync.dma_start(cc_in[:], pooled_sb[:])
            if sim_mode:
                nc.sync.dma_start(cc_out[:], cc_in[:])
            else:
                nc.gpsimd.collective_compute(
                    "AllReduce",
                    ALU.add,
                    replica_groups=[[0, 1], [2, 3], [4, 5], [6, 7]],
                    ins=[cc_in.opt()],
                    outs=[cc_out.opt()],
                )
            pooled_f = work_pool.tile([S_TOT, 512], f32, tag="pooled", name="pooled_f", bufs=1)
            nc.sync.dma_start(pooled_f[:], cc_out[:])

            # tile 0's bottleneck-conv matmuls: independent of the collective,
            # keep the PE busy during the AllReduce round trip
            def emit_bf(t, m, op):
                px = bass.ts(t, PIX_T)
                nc.tensor.matmul(op[:], wbf_ap[m][0], f8a3[:, :, px],
                                 start=True, stop=False, perf_mode=DR)
                nc.tensor.matmul(op[:], wbf_ap[m][1], f8b3[:, :, px],
                                 start=False, stop=False, perf_mode=DR)
                nc.tensor.matmul(op[:], wbfr_ap[m][0], f8a3[:, :, px],
                                 start=False, stop=False, perf_mode=DR)
                nc.tensor.matmul(op[:], wbfr_ap[m][1], f8b3[:, :, px],
                                 start=False, stop=False, perf_mode=DR)
                nc.tensor.matmul(op[:], wbf_ap[m][0], fr8a3[:, :, px],
                                 start=False, stop=False, perf_mode=DR)
                nc.tensor.matmul(op[:], wbf_ap[m][1], fr8b3[:, :, px],
                                 start=False, stop=False, perf_mode=DR)

            # out psums rotate over 5 banks: the retired pooled bank + pbig's 4
            def alloc_ops():
                return [pacc.tile([128, PIX_T], f32, tag="pooled", name="op",
                                  padded_shape=[128, 512])] + \
                       [pbig.tile([128, PIX_T], f32, tag="big", name="op")
                        for _ in range(3)]

            ops0 = alloc_ops()
            for m in range(4):
                emit_bf(0, m, ops0[m])

            # keyval8: [110, 512] fp8 = [32*keys*sc | 32*v_pool]
            keyval8 = const_pool.tile([S_TOT, 512], f8, tag="keyval", name="keyval8")
            nc.vector.tensor_scalar(
                keyval8[:, :CK], pooled_f[:, :CK], iak_ap, None, ALU.mult)
            nc.vector.tensor_scalar(
                keyval8[:, CK:], pooled_f[:, CK:], iav_ap, None, ALU.mult)
            rot.add("dve", 700)

            # key_cm / v_cm channel-major pair layouts [128, 2*110]
            kt = p1k.tile([128, 2 * S_TOT], f8, tag="t1k", name="kt",
                          padded_shape=[128, 1024])
            for mc in range(2):
                nc.tensor.transpose(
                    kt[:, mc * S_TOT:(mc + 1) * S_TOT],
                    keyval8[:, mc * 128:(mc + 1) * 128],
                    ident8[:S_TOT, :S_TOT])
            key_cm = const_pool.tile([128, 2 * S_TOT], f8, tag="keycm", name="key_cm")
            nc.scalar.copy(key_cm[:], kt[:])
            rot.add("act", 400)
            key3 = key_cm[:].rearrange("k (i s) -> k i s", i=2)

            vt = p1k.tile([128, 2 * S_TOT], f8, tag="t1k", name="vt",
                          padded_shape=[128, 1024])
            for mc in range(2):
                nc.tensor.transpose(
                    vt[:, mc * S_TOT:(mc + 1) * S_TOT],
                    keyval8[:, CK + mc * 128:CK + (mc + 1) * 128],
                    ident8[:S_TOT, :S_TOT])
            v_cm = const_pool.tile([128, 2 * S_TOT], f8, tag="vcm", name="v_cm")
            nc.scalar.copy(v_cm[:], vt[:])
            rot.add("act", 400)
            v3 = v_cm[:].rearrange("k (i s) -> k i s", i=2)

            # u_pool = W_co @ v_pool (DR), drained to fp8 at 32x
            u_ps = psim.tile([128, 4 * S_TOT], f32, tag="sim", name="u_ps",
                             padded_shape=[128, 512])
            for o in range(4):
                nc.tensor.matmul(u_ps[:, o * S_TOT:(o + 1) * S_TOT],
                                 wco_ap[o], v3, start=True, stop=True,
                                 perf_mode=DR)
            u_sb = const_pool.tile([128, 4 * S_TOT], f8, tag="usb", name="u_sb")
            nc.vector.tensor_scalar(u_sb[:], u_ps[:], U_DRAIN, None, ALU.mult)
            rot.add("dve", 600)

            # u_poolT in [55, 2, 128] DR layout per o-chunk
            upT_ps = p1k.tile([S_HALF, 4 * 256], f8, tag="t1k", name="upT_ps",
                              padded_shape=[128, 1024])
            for o in range(4):
                for h in range(2):
                    nc.tensor.transpose(
                        upT_ps[:, o * 256 + h * 128:o * 256 + (h + 1) * 128],
                        u_sb[:, o * S_TOT + h * S_HALF:o * S_TOT + (h + 1) * S_HALF],
                        ident8[:])
            upT_sb = const_pool.tile([S_HALF, 4 * 256], f8, tag="upT", name="upT_sb")
            nc.scalar.copy(upT_sb[:], upT_ps[:])
            rot.add("act", 1000)
            upT_ap = [upT_sb[:, o * 256:(o + 1) * 256].rearrange(
                "s (i m) -> s i m", i=2) for o in range(4)]

            # ---- phase D: attention + fused output (software pipelined) ----
            def emit_sim(t):
                sim = psim.tile([128, 4 * S_TOT], f32, tag="sim", name="sim",
                                padded_shape=[128, 512])
                for u in range(4):
                    upx = bass.ts(t * 4 + u, 128)
                    nc.tensor.matmul(
                        sim[:, u * S_TOT:(u + 1) * S_TOT],
                        q83[:, :, upx], key3, start=True, stop=True,
                        perf_mode=DR)
                attn_f = work_pool.tile([128, 4 * S_TOT], f32, tag="attnf", name="attn_f")
                nc.scalar.activation(attn_f[:], sim[:], AF.Exp, scale=EXP_SCALE)
                rot.add("act", 750)
                den = work_pool.tile([128, 4], f32, tag="den", name="den")
                nc.vector.tensor_reduce(
                    den[:], attn_f[:].rearrange("p (u s) -> p u s", s=S_TOT),
                    axis=mybir.AxisListType.X, op=ALU.add)
                rden = work_pool.tile([128, 4], f32, tag="rden", name="rden")
                nc.vector.reciprocal(rden[:], den[:])
                attn8 = work_pool.tile([128, 4 * S_TOT], f8, tag="attn8", name="attn8")
                for u in range(4):
                    usl = bass.ts(u, S_TOT)
                    nc.vector.tensor_scalar(
                        attn8[:, usl], attn_f[:, usl], rden[:, u:u + 1],
                        S_ATTN, ALU.mult, ALU.mult)
                rot.add("dve", 1100)
                return attn8

            def emit_attnT(t, attn8):
                aTp = p1k.tile([S_HALF, 1024], f8, tag="t1k", name="aTp",
                               padded_shape=[128, 1024])
                for u in range(4):
                    for h in range(2):
                        nc.tensor.transpose(
                            aTp[:, h * 512 + u * 128:h * 512 + (u + 1) * 128],
                            attn8[:, u * S_TOT + h * S_HALF:
                                  u * S_TOT + (h + 1) * S_HALF],
                            ident8[:])
                # latency-critical: keep on DVE so ctx isn't queued behind
                # slower engines
                aT_sb = work_pool.tile([S_HALF, 1024], f8, tag="aTsb", name="aT_sb")
                nc.vector.tensor_copy(aT_sb[:, :512], aTp[:, :512])
                nc.vector.tensor_copy(aT_sb[:, 512:], aTp[:, 512:])
                rot.add("dve", 1320)
                return aT_sb

            def emit_ctx(t, m, op, aT_sb):
                px = bass.ts(t, PIX_T)
                aT3 = aT_sb[:].rearrange("s (i n) -> s i n", i=2)
                nc.tensor.matmul(op[:], upT_ap[m], aT3,
                                 start=False, stop=True, perf_mode=DR)
                o_sb = out_pool.tile([128, PIX_T], bf16, tag="osb", name="o_sb")
                relu_drain(o_sb[:], op[:], bco_ap[m], PIX_T, engines=("act", "pool"))
                nc.sync.dma_start(out_d[m * 128:(m + 1) * 128, px], o_sb[:])

            attn8_cur = emit_sim(0)
            for t in range(N_T):
                aT_sb = emit_attnT(t, attn8_cur)
                attn8_next = emit_sim(t + 1) if t + 1 < N_T else None
                if t == 0:
                    ops = ops0
                else:
                    ops = alloc_ops()
                    for m in range(4):
                        emit_bf(t, m, ops[m])
                for m in range(4):
                    emit_ctx(t, m, ops[m], aT_sb)
                attn8_cur = attn8_next

    nc.compile()
    return nc


_NC_CACHE = {}
TRACE = False
LAST_RESULT = {}


def kernel(feats, w_kq, b_kq, w_v, b_v, w_out, b_out, w_bot, b_bot):
    from concourse.bass_utils import run_bass_kernel_spmd

    feats = np.asarray(feats, dtype=np.float32)
    w_kq = np.asarray(w_kq, dtype=np.float32)
    b_kq = np.asarray(b_kq, dtype=np.float32)
    w_v = np.asarray(w_v, dtype=np.float32)
    b_v = np.asarray(b_v, dtype=np.float32)
    w_out = np.asarray(w_out, dtype=np.float32)
    b_out = np.asarray(b_out, dtype=np.float32)
    w_bot = np.asarray(w_bot, dtype=np.float32)
    b_bot = np.asarray(b_bot, dtype=np.float32)

    # host-side weight prep
    w_co = w_bot[:, :C] @ w_out                     # [CO, CK]
    b_co = w_bot[:, :C] @ b_out + b_bot + w_co @ b_v
    w_bf = w_bot[:, C:]                             # [CO, C]

    # wq8/wv8: [k, mc*512 + p*256 + i*128 + m] = 64*w[mc*128+m, (2p+i)*128+k]
    def proj_pack(wm):
        arr = wm.T.reshape(2, 2, 128, 2, 128)       # [p, i, k, mc, m]
        return _f8(arr.transpose(2, 3, 0, 1, 4).reshape(128, 1024), S_W)

    wq8 = proj_pack(w_kq)
    wv8 = proj_pack(w_v)

    # wbf8: [k, p*1024 + i*512 + o] = 2048*w_bf[o, (2p+i)*128+k]
    arr = w_bf.T.reshape(2, 2, 128, CO)             # [p, i, k, o]
    wbf_lay = arr.transpose(2, 0, 1, 3).reshape(128, 2048)
    wbf8 = _f8(wbf_lay, S_BF)
    wbfr8 = _f8(wbf_lay - wbf8.astype(np.float32) / np.float32(S_BF), S_BF)

    # wco8: [k, i*512 + o] = 64*w_co[o, i*128+k]
    arr = w_co.T.reshape(2, 128, CO)                # [i, k, o]
    wco8 = _f8(arr.transpose(1, 0, 2).reshape(128, 1024), S_WCO)

    if "nc" not in _NC_CACHE:
        _NC_CACHE["nc"] = _build_kernel()
    nc = _NC_CACHE["nc"]

    sc = np.float32(1.0 / np.sqrt(CK))
    in_maps = []
    for core in range(N_CORES):
        b, half = core // 2, core % 2
        h0 = half * H_SH
        pmat, inv_area = _build_pool_mat(h0)
        biases = np.zeros((128, 8), dtype=np.float32)
        biases[:, 0:2] = (S_W * b_kq).reshape(2, 128).T
        biases[:, 2:6] = (S_BF * b_co).reshape(4, 128).T
        biases[:S_TOT, 6] = inv_area * sc * (S_KEY / S_Q)
        biases[:S_TOT, 7] = inv_area * (S_VP / S_Q)
        shard = np.ascontiguousarray(
            feats[b, :, h0:h0 + H_SH, :]).reshape(C, NPIX)
        f8v = _f8(shard).astype(np.float32)
        fr8v = _f8(shard - f8v)
        f8v = f8v.astype(F8)
        # pair layouts [128, 2*NPIX]: chunk-pair planes side by side
        c4 = f8v.reshape(4, 128, NPIX)
        r4 = fr8v.reshape(4, 128, NPIX)
        f8cat = np.concatenate(
            [np.concatenate([c4[0], c4[1]], axis=1),
             np.concatenate([c4[2], c4[3]], axis=1),
             np.concatenate([r4[0], r4[1]], axis=1),
             np.concatenate([r4[2], r4[3]], axis=1)], axis=1)
        in_maps.append(dict(
            f8cat=np.ascontiguousarray(f8cat), wq8=wq8, wv8=wv8,
            wbf8=wbf8, wbfr8=wbfr8, wco8=wco8,
            pmat=pmat.reshape(W, H_SH * S_TOT), biases=biases,
        ))

    res = run_bass_kernel_spmd(
        nc, in_maps, list(range(N_CORES)), trace=TRACE
    )
    LAST_RESULT["res"] = res

    out = np.empty((B, CO, H, W), dtype=np.float32)
    for core in range(N_CORES):
        b, half = core // 2, core % 2
        h0 = half * H_SH
        out[b, :, h0:h0 + H_SH, :] = (
            np.asarray(res.results[core]["out"]).astype(np.float32)
            * OUT_UNSCALE
        ).reshape(CO, H_SH, W)
    return out


# revision 22
# speedup vs baseline: 1.7301x; 1.0351x over previous
"""APNB (asymmetric pyramid non-local block) Trainium2 kernel, fp8 edition.

Full inputs in, full output out. Sharding: 8 cores = (batch b=core//2,
H-half=core%2). Each core handles feats[b, :, 64*half:64*half+64, :]
(= [512, 8192] pixels).

All heavy GEMMs run as fp8-e4m3 DoubleRow matmuls (2 k-chunks per
instruction). The precision-critical bottleneck conv on feats uses a
3-term fp8 residual decomposition (w8@f8 + w8@fr8 + wr8@f8, shared
power-of-2 scale 2048) which matches bf16 accuracy. The attention side
(q/v projections, PPM pooling, softmax, pooled-value contraction with
W_out folded into W_bot on the host) is plain fp8 with per-tensor
power-of-2 scales. Final output is produced at 2048x scale and
exponent-shifted back on the host (exact).

Engine budget: elementwise drains/copies are load-balanced across
Vector/Scalar/GpSimd with a greedy cost tracker; phases A/B are
interleaved and phase D is software-pipelined so the PE never starves.
"""

import sys

for p in ("/opt/trn_rl_repo",):
    if p not in sys.path:
        sys.path.insert(0, p)

import numpy as np
import ml_dtypes

BF16 = ml_dtypes.bfloat16
F8 = ml_dtypes.float8_e4m3  # TRN float8e4 (max +-240)

# ---- problem constants (hardcoded per spec) ----
B, C, H, W = 4, 512, 128, 128
CK, CO = 256, 512
POOL_SCALES = (1, 3, 6, 8)
S_TOT = sum(s * s for s in POOL_SCALES)  # 110
S_HALF = S_TOT // 2                      # 55
N_CORES = 8
H_SH = H // 2          # 64 rows per core
NPIX = H_SH * W        # 8192 pixels per core
PIX_T = 512            # gemm pixel tile
N_T = NPIX // PIX_T    # 16
N_PAIR = H_SH // 2     # 32 row pairs for DoubleRow pooling

# power-of-2 scales
S_W = 64.0        # w_kq / w_v
S_Q = 64.0        # q8 = 64*relu(q), v8 = 64*v
S_KEY = 32.0      # keys
S_VP = 32.0       # pooled v (channel-major)
S_WCO = 64.0      # folded W_co
S_UP = 32.0       # u_pool
S_ATTN = 64.0     # attn weights
S_BF = 2048.0     # bottleneck feats weight + out psum scale
EXP_SCALE = 1.0 / (S_Q * S_KEY)   # fold q/key scales out inside exp
U_DRAIN = S_UP / (S_WCO * S_VP)   # u psum -> u8
OUT_UNSCALE = np.float32(1.0 / S_BF)


def _pool_bounds(n, s):
    i = np.arange(s)
    return (i * n) // s, -((-(i + 1) * n) // s)


def _build_pool_mat(h0):
    """P[w, r, bin] = 1 if global pixel (h0+r, w) is in bin; fp8 (exact)."""
    P = np.zeros((W, H_SH, S_TOT), dtype=np.float32)
    inv_area = np.zeros((S_TOT,), dtype=np.float32)
    off = 0
    for s in POOL_SCALES:
        hs, he = _pool_bounds(H, s)
        ws, we = _pool_bounds(W, s)
        for i in range(s):
            for j in range(s):
                b = off + i * s + j
                inv_area[b] = 1.0 / float((he[i] - hs[i]) * (we[j] - ws[j]))
                r0 = max(hs[i] - h0, 0)
                r1 = min(he[i] - h0, H_SH)
                if r1 > r0:
                    P[ws[j]:we[j], r0:r1, b] = 1.0
        off += s * s
    return P.astype(F8), inv_area


def _f8(x, scale=1.0):
    y = np.asarray(x, np.float32) * np.float32(scale)
    y = np.clip(y, -240.0, 240.0)
    return y.astype(F8)


def _build_kernel(sim_mode=False):
    import concourse.bass as bass
    import concourse.bacc as bacc
    import concourse.mybir as mybir
    from concourse import tile, masks

    dt = mybir.dt
    f32, bf16, f8 = dt.float32, dt.bfloat16, dt.float8e4
    AF = mybir.ActivationFunctionType
    ALU = mybir.AluOpType
    DR = mybir.MatmulPerfMode.DoubleRow

    nc = bacc.Bacc("TRN2", target_bir_lowering=False, debug=False)

    # ---- DRAM I/O ----
    # f8cat: [f8a | f8b | fr8a | fr8b], each [128, 2*NPIX] chunk-pair layout
    f8cat_d = nc.dram_tensor("f8cat", [128, 8 * NPIX], f8, kind="ExternalInput")
    wq8_d = nc.dram_tensor("wq8", [128, 2 * 512], f8, kind="ExternalInput")
    wv8_d = nc.dram_tensor("wv8", [128, 2 * 512], f8, kind="ExternalInput")
    wbf8_d = nc.dram_tensor("wbf8", [128, 2048], f8, kind="ExternalInput")
    wbfr8_d = nc.dram_tensor("wbfr8", [128, 2048], f8, kind="ExternalInput")
    wco8_d = nc.dram_tensor("wco8", [128, 1024], f8, kind="ExternalInput")
    pmat_d = nc.dram_tensor("pmat", [W, H_SH * S_TOT], f8, kind="ExternalInput")
    bias_d = nc.dram_tensor("biases", [128, 8], f32, kind="ExternalInput")
    out_d = nc.dram_tensor("out", [CO, NPIX], bf16, kind="ExternalOutput")

    # greedy engine load balancer for drains/copies
    class Rot:
        def __init__(self):
            self.load = {"dve": 0.0, "act": 0.0, "pool": 0.0}

        def add(self, e, ns):
            self.load[e] += ns

        def pick(self, width, psum_in=True):
            return self.pick2(width, ("dve", "act", "pool"), psum_in)

        def pick2(self, width, engines, psum_in=True):
            cost = {
                "dve": width * 1.05 + (130.0 if psum_in else 65.0),
                "act": width * 0.84 + 190.0,
                "pool": width * 1.39 + 125.0,
            }
            best = min((self.load[e] + cost[e], cost[e], e) for e in engines)
            self.load[best[2]] = best[0]
            return best[2]

    rot = Rot()

    with tile.TileContext(nc) as tc:
        with (
            tc.tile_pool(name="const", bufs=1) as const_pool,
            tc.tile_pool(name="work", bufs=2) as work_pool,
            tc.tile_pool(name="outb", bufs=4) as out_pool,
            tc.tile_pool(name="pbig", bufs=4, space="PSUM") as pbig,
            tc.tile_pool(name="pacc", bufs=1, space="PSUM") as pacc,
            tc.tile_pool(name="p1k", bufs=2, space="PSUM") as p1k,
            tc.tile_pool(name="psim", bufs=1, space="PSUM") as psim,
            tc.tile_pool(name="dram", bufs=1, space="DRAM") as dram_pool,
        ):
            def relu_drain(out, in_, bias_ap, width, engines=None):
                e = rot.pick(width) if engines is None else rot.pick2(width, engines)
                if e == "dve":
                    nc.vector.tensor_scalar(out, in_, bias_ap, 0.0, ALU.add, ALU.max)
                elif e == "act":
                    nc.scalar.activation(out, in_, AF.Relu, bias=bias_ap)
                else:
                    nc.gpsimd.tensor_scalar(out, in_, bias_ap, 0.0, ALU.add, ALU.max)

            def copy_drain(out, in_, width):
                e = rot.pick(width)
                if e == "dve":
                    nc.vector.tensor_copy(out, in_)
                elif e == "act":
                    nc.scalar.copy(out, in_)
                else:
                    nc.gpsimd.tensor_copy(out, in_)

            # ---- constants / weights ----
            ident8 = const_pool.tile([128, 128], f8, tag="ident", name="ident8")
            masks.make_identity(nc, ident8[:])

            # feats fp8 (pair layouts) + residuals
            f8a = const_pool.tile([128, 2 * NPIX], f8, tag="f8a", name="f8a")
            f8b = const_pool.tile([128, 2 * NPIX], f8, tag="f8b", name="f8b")
            fr8a = const_pool.tile([128, 2 * NPIX], f8, tag="fr8a", name="fr8a")
            fr8b = const_pool.tile([128, 2 * NPIX], f8, tag="fr8b", name="fr8b")
            f8a3 = f8a[:].rearrange("k (i n) -> k i n", i=2)
            f8b3 = f8b[:].rearrange("k (i n) -> k i n", i=2)
            fr8a3 = fr8a[:].rearrange("k (i n) -> k i n", i=2)
            fr8b3 = fr8b[:].rearrange("k (i n) -> k i n", i=2)

            wq8_sb = const_pool.tile([128, 1024], f8, tag="wq8", name="wq8_sb")
            wv8_sb = const_pool.tile([128, 1024], f8, tag="wv8", name="wv8_sb")
            bias_sb = const_pool.tile([128, 8], f32, tag="bias", name="bias_sb")
            pmat_sb = const_pool.tile([W, H_SH * S_TOT], f8, tag="pmat", name="pmat_sb")
            wbf8_sb = const_pool.tile([128, 2048], f8, tag="wbf8", name="wbf8_sb")
            wbfr8_sb = const_pool.tile([128, 2048], f8, tag="wbfr8", name="wbfr8_sb")
            wco8_sb = const_pool.tile([128, 1024], f8, tag="wco8", name="wco8_sb")

            def dma_f8_slices(t):
                g = t // 2
                c0 = g * 2 * PIX_T
                nc.sync.dma_start(f8a[:, c0:c0 + 1024], f8cat_d[:, c0:c0 + 1024])
                nc.sync.dma_start(
                    f8a[:, NPIX + c0:NPIX + c0 + 1024],
                    f8cat_d[:, NPIX + c0:NPIX + c0 + 1024])
                nc.sync.dma_start(
                    f8b[:, c0:c0 + 1024],
                    f8cat_d[:, 2 * NPIX + c0:2 * NPIX + c0 + 1024])
                nc.sync.dma_start(
                    f8b[:, NPIX + c0:NPIX + c0 + 1024],
                    f8cat_d[:, 3 * NPIX + c0:3 * NPIX + c0 + 1024])

            # first feats slices, then small consts
            dma_f8_slices(0)
            nc.sync.dma_start(wq8_sb[:], wq8_d[:])
            nc.sync.dma_start(wv8_sb[:], wv8_d[:])
            nc.sync.dma_start(bias_sb[:], bias_d[:])

            bkq_ap = [bias_sb[:, m:m + 1] for m in range(2)]            # 64*b_kq
            bco_ap = [bias_sb[:, 2 + m:3 + m] for m in range(4)]        # 2048*b_co
            iak_ap = bias_sb[:S_TOT, 6:7]                               # inv_area*sc/2
            iav_ap = bias_sb[:S_TOT, 7:8]                               # inv_area/2

            pmat3 = pmat_sb[:].rearrange("w (r s) -> w r s", s=S_TOT)

            # q8 = 64*relu(q); [128, 2*NPIX]: m-chunk planes
            q8 = const_pool.tile([128, 2 * NPIX], f8, tag="q8", name="q8")
            q83 = q8[:].rearrange("k (i n) -> k i n", i=2)

            # DR weight APs
            def pair_ap(tile_ap, base, width):
                return tile_ap[:, base:base + 2 * width].rearrange(
                    "k (i m) -> k i m", i=2)

            wq_ap = [[pair_ap(wq8_sb[:], mc * 512 + p * 256, 128)
                      for p in range(2)] for mc in range(2)]
            # wv in moving layout for direct v^T: [k, p*512 + i*256 + o]
            wv_ap = [pair_ap(wv8_sb[:], p * 512, 256) for p in range(2)]
            wbf_ap = [[pair_ap(wbf8_sb[:], p * 1024, 512)[:, :, m * 128:(m + 1) * 128]
                       for p in range(2)] for m in range(4)]
            wbfr_ap = [[pair_ap(wbfr8_sb[:], p * 1024, 512)[:, :, m * 128:(m + 1) * 128]
                        for p in range(2)] for m in range(4)]
            wco_ap = [pair_ap(wco8_sb[:], 0, 512)[:, :, o * 128:(o + 1) * 128]
                      for o in range(4)]

            # ---- phases A+B interleaved ----
            pooled_ps = pacc.tile([S_TOT, 512], f32, tag="pooled", name="pooled_ps")
            qvT_list = []

            def emit_a_tile(t):
                px = bass.ts(t, PIX_T)
                for mc in range(2):
                    qp = pbig.tile([128, PIX_T], f32, tag="big", name="qp")
                    nc.tensor.matmul(qp[:], wq_ap[mc][0], f8a3[:, :, px],
                                     start=True, stop=False, perf_mode=DR)
                    nc.tensor.matmul(qp[:], wq_ap[mc][1], f8b3[:, :, px],
                                     start=False, stop=True, perf_mode=DR)
                    relu_drain(q8[:, mc * NPIX + t * PIX_T:
                                  mc * NPIX + (t + 1) * PIX_T],
                               qp[:], bkq_ap[mc], PIX_T)

            def emit_b_pair(p):
                # v^T computed directly (feats stationary): [128 pix, 256]
                qvT = work_pool.tile([128, 1024], f8, tag="qvT", name="qvT", bufs=3)
                qvT3 = qvT[:].rearrange("w (j c) -> w j c", j=2)
                if p % 2 == 0:
                    vp2 = psim.tile([128, 512], f32, tag="sim", name="vp2")
                else:
                    vp2 = pbig.tile([128, 512], f32, tag="big", name="vp2")
                for j in range(2):
                    rpx = bass.ts(2 * p + j, 128)
                    nc.tensor.matmul(vp2[:, j * 256:(j + 1) * 256],
                                     f8a3[:, :, rpx], wv_ap[0],
                                     start=True, stop=False, perf_mode=DR)
                    nc.tensor.matmul(vp2[:, j * 256:(j + 1) * 256],
                                     f8b3[:, :, rpx], wv_ap[1],
                                     start=False, stop=True, perf_mode=DR)
                copy_drain(qvT3[:, :, 256:512],
                           vp2[:].rearrange("w (j c) -> w j c", j=2), PIX_T)
                rowT = p1k.tile([128, 512], f8, tag="t1k", name="rowT",
                                padded_shape=[128, 1024])
                for j in range(2):
                    r = 2 * p + j
                    for mc in range(2):
                        nc.tensor.transpose(
                            rowT[:, j * 256 + mc * 128:j * 256 + (mc + 1) * 128],
                            q8[:, mc * NPIX + r * 128:mc * NPIX + (r + 1) * 128],
                            ident8[:])
                copy_drain(qvT3[:, :, 0:256],
                           rowT[:].rearrange("w (j c) -> w j c", j=2), PIX_T)
                qvT_list.append(qvT)

            def emit_pool_mm(pp):
                nc.tensor.matmul(
                    pooled_ps[:], pmat3[:, 2 * pp:2 * pp + 2, :],
                    qvT_list[pp][:].rearrange("w (i c) -> w i c", i=2),
                    start=(pp == 0), stop=(pp == N_PAIR - 1), perf_mode=DR)

            for t in range(N_T):
                if t >= 2 and t % 2 == 0:
                    dma_f8_slices(t)
                if 1 <= t <= 4:   # pmat in 4 row-chunks (16 rows each)
                    g = t - 1
                    nc.sync.dma_start(
                        pmat_sb[:, g * 16 * S_TOT:(g + 1) * 16 * S_TOT],
                        pmat_d[:, g * 16 * S_TOT:(g + 1) * 16 * S_TOT])
                if 5 <= t <= 12:  # fr8 in 8 column-chunks
                    g = t - 5
                    base = 4 * NPIX + g * 2048
                    dst = fr8a if g < 4 else fr8b
                    off = (g % 4) * 2048
                    nc.sync.dma_start(dst[:, off:off + 2048],
                                      f8cat_d[:, base:base + 2048])
                if t == 13:
                    nc.sync.dma_start(wbf8_sb[:], wbf8_d[:])
                if t == 14:
                    nc.sync.dma_start(wbfr8_sb[:], wbfr8_d[:])
                if t == 15:
                    nc.sync.dma_start(wco8_sb[:], wco8_d[:])
                emit_a_tile(t)
                if t >= 2:
                    for p in (2 * (t - 2), 2 * (t - 2) + 1):
                        emit_b_pair(p)
                        if p >= 2:
                            emit_pool_mm(p - 2)
            for p in range(2 * (N_T - 2), N_PAIR):
                emit_b_pair(p)
                if p >= 2:
                    emit_pool_mm(p - 2)
            for pp in range(N_PAIR - 2, N_PAIR):
                emit_pool_mm(pp)

            # ---- phase C: AllReduce + pooled-side prep ----
            pooled_sb = work_pool.tile([S_TOT, 512], f32, tag="pooled", name="pooled_sb", bufs=1)
            nc.vector.tensor_copy(pooled_sb[:], pooled_ps[:])
            rot.add("dve", 700)
            cc_in = dram_pool.tile([S_TOT, 512], f32, tag="cc_in", name="cc_in")
            cc_out = dram_pool.tile([S_TOT, 512], f32, tag="cc_out", name="cc_out")
            nc.sync.dma_start(cc_in[:], pooled_sb[:])
            if sim_mode:
                nc.sync.dma_start(cc_out[:], cc_in[:])
            else:
                nc.gpsimd.collective_compute(
                    "AllReduce",
                    ALU.add,
                    replica_groups=[[0, 1], [2, 3], [4, 5], [6, 7]],
                    ins=[cc_in.opt()],
                    outs=[cc_out.opt()],
                )
            pooled_f = work_pool.tile([S_TOT, 512], f32, tag="pooled", name="pooled_f", bufs=1)
            nc.sync.dma_start(pooled_f[:], cc_out[:])

            # tile 0's bottleneck-conv matmuls: independent of the collective,
            # keep the PE busy during the AllReduce round trip
            def emit_bf(t, m, op):
                px = bass.ts(t, PIX_T)
                nc.tensor.matmul(op[:], wbf_ap[m][0], f8a3[:, :, px],
                                 start=True, stop=False, perf_mode=DR)
                nc.tensor.matmul(op[:], wbf_ap[m][1], f8b3[:, :, px],
                                 start=False, stop=False, perf_mode=DR)
                nc.tensor.matmul(op[:], wbfr_ap[m][0], f8a3[:, :, px],
                                 start=False, stop=False, perf_mode=DR)
                nc.tensor.matmul(op[:], wbfr_ap[m][1], f8b3[:, :, px],
                                 start=False, stop=False, perf_mode=DR)
                nc.tensor.matmul(op[:], wbf_ap[m][0], fr8a3[:, :, px],
                                 start=False, stop=False, perf_mode=DR)
                nc.tensor.matmul(op[:], wbf_ap[m][1], fr8b3[:, :, px],
                                 start=False, stop=False, perf_mode=DR)

            # out psums rotate over 5 banks: the retired pooled bank + pbig's 4
            def alloc_ops():
                return [pacc.tile([128, PIX_T], f32, tag="pooled", name="op",
                                  padded_shape=[128, 512])] + \
                       [pbig.tile([128, PIX_T], f32, tag="big", name="op")
                        for _ in range(3)]

            ops0 = alloc_ops()
            for m in range(4):
                emit_bf(0, m, ops0[m])

            # keyval8: [110, 512] fp8 = [32*keys*sc | 32*v_pool]
            keyval8 = const_pool.tile([S_TOT, 512], f8, tag="keyval", name="keyval8")
            nc.vector.tensor_scalar(
                keyval8[:, :CK], pooled_f[:, :CK], iak_ap, None, ALU.mult)
            nc.vector.tensor_scalar(
                keyval8[:, CK:], pooled_f[:, CK:], iav_ap, None, ALU.mult)
            rot.add("dve", 700)

            # key_cm / v_cm channel-major pair layouts [128, 2*110]
            kt = p1k.tile([128, 2 * S_TOT], f8, tag="t1k", name="kt",
                          padded_shape=[128, 1024])
            for mc in range(2):
                nc.tensor.transpose(
                    kt[:, mc * S_TOT:(mc + 1) * S_TOT],
                    keyval8[:, mc * 128:(mc + 1) * 128],
                    ident8[:S_TOT, :S_TOT])
            key_cm = const_pool.tile([128, 2 * S_TOT], f8, tag="keycm", name="key_cm")
            nc.scalar.copy(key_cm[:], kt[:])
            rot.add("act", 400)
            key3 = key_cm[:].rearrange("k (i s) -> k i s", i=2)

            vt = p1k.tile([128, 2 * S_TOT], f8, tag="t1k", name="vt",
                          padded_shape=[128, 1024])
            for mc in range(2):
                nc.tensor.transpose(
                    vt[:, mc * S_TOT:(mc + 1) * S_TOT],
                    keyval8[:, CK + mc * 128:CK + (mc + 1) * 128],
                    ident8[:S_TOT, :S_TOT])
            v_cm = const_pool.tile([128, 2 * S_TOT], f8, tag="vcm", name="v_cm")
            nc.scalar.copy(v_cm[:], vt[:])
            rot.add("act", 400)
            v3 = v_cm[:].rearrange("k (i s) -> k i s", i=2)

            # u_pool = W_co @ v_pool (DR), drained to fp8 at 32x
            u_ps = psim.tile([128, 4 * S_TOT], f32, tag="sim", name="u_ps",
                             padded_shape=[128, 512])
            for o in range(4):
                nc.tensor.matmul(u_ps[:, o * S_TOT:(o + 1) * S_TOT],
                                 wco_ap[o], v3, start=True, stop=True,
                                 perf_mode=DR)
            u_sb = const_pool.tile([128, 4 * S_TOT], f8, tag="usb", name="u_sb")
            nc.vector.tensor_scalar(u_sb[:], u_ps[:], U_DRAIN, None, ALU.mult)
            rot.add("dve", 600)

            # u_poolT in [55, 2, 128] DR layout per o-chunk
            upT_ps = p1k.tile([S_HALF, 4 * 256], f8, tag="t1k", name="upT_ps",
                              padded_shape=[128, 1024])
            for o in range(4):
                for h in range(2):
                    nc.tensor.transpose(
                        upT_ps[:, o * 256 + h * 128:o * 256 + (h + 1) * 128],
                        u_sb[:, o * S_TOT + h * S_HALF:o * S_TOT + (h + 1) * S_HALF],
                        ident8[:])
            upT_sb = const_pool.tile([S_HALF, 4 * 256], f8, tag="upT", name="upT_sb")
            nc.scalar.copy(upT_sb[:], upT_ps[:])
            rot.add("act", 1000)
            upT_ap = [upT_sb[:, o * 256:(o + 1) * 256].rearrange(
                "s (i m) -> s i m", i=2) for o in range(4)]

            # ---- phase D: attention + fused output (software pipelined) ----
            def emit_sim(t):
                sim = psim.tile([128, 4 * S_TOT], f32, tag="sim", name="sim",
                                padded_shape=[128, 512])
                for u in range(4):
                    upx = bass.ts(t * 4 + u, 128)
                    nc.tensor.matmul(
                        sim[:, u * S_TOT:(u + 1) * S_TOT],
                        q83[:, :, upx], key3, start=True, stop=True,
                        perf_mode=DR)
                attn_f = work_pool.tile([128, 4 * S_TOT], f32, tag="attnf", name="attn_f")
                nc.scalar.activation(attn_f[:], sim[:], AF.Exp, scale=EXP_SCALE)
                rot.add("act", 750)
                den = work_pool.tile([128, 4], f32, tag="den", name="den")
                nc.vector.tensor_reduce(
                    den[:], attn_f[:].rearrange("p (u s) -> p u s", s=S_TOT),
                    axis=mybir.AxisListType.X, op=ALU.add)
                rden = work_pool.tile([128, 4], f32, tag="rden", name="rden")
                nc.vector.reciprocal(rden[:], den[:])
                attn8 = work_pool.tile([128, 4 * S_TOT], f8, tag="attn8", name="attn8")
                for u in range(4):
                    usl = bass.ts(u, S_TOT)
                    nc.vector.tensor_scalar(
                        attn8[:, usl], attn_f[:, usl], rden[:, u:u + 1],
                        S_ATTN, ALU.mult, ALU.mult)
                rot.add("dve", 1100)
                return attn8

            def emit_attnT(t, attn8):
                aTp = p1k.tile([S_HALF, 1024], f8, tag="t1k", name="aTp",
                               padded_shape=[128, 1024])
                for u in range(4):
                    for h in range(2):
                        nc.tensor.transpose(
                            aTp[:, h * 512 + u * 128:h * 512 + (u + 1) * 128],
                            attn8[:, u * S_TOT + h * S_HALF:
                                  u * S_TOT + (h + 1) * S_HALF],
                            ident8[:])
                # latency-critical: keep on DVE so ctx isn't queued behind
                # slower engines
                aT_sb = work_pool.tile([S_HALF, 1024], f8, tag="aTsb", name="aT_sb")
                nc.vector.tensor_copy(aT_sb[:, :512], aTp[:, :512])
                nc.vector.tensor_copy(aT_sb[:, 512:], aTp[:, 512:])
                rot.add("dve", 1320)
                return aT_sb

            def emit_ctx(t, m, op, aT_sb):
                px = bass.ts(t, PIX_T)
                aT3 = aT_sb[:].rearrange("s (i n) -> s i n", i=2)
                nc.tensor.matmul(op[:], upT_ap[m], aT3,
                                 start=False, stop=True, perf_mode=DR)
                o_sb = out_pool.tile([128, PIX_T], bf16, tag="osb", name="o_sb")
                relu_drain(o_sb[:], op[:], bco_ap[m], PIX_T, engines=("act", "pool"))
                nc.sync.dma_start(out_d[m * 128:(m + 1) * 128, px], o_sb[:])

            attn8_cur = emit_sim(0)
            for t in range(N_T):
                aT_sb = emit_attnT(t, attn8_cur)
                attn8_next = emit_sim(t + 1) if t + 1 < N_T else None
                if t == 0:
                    ops = ops0
                else:
                    ops = alloc_ops()
                    for m in range(4):
                        emit_bf(t, m, ops[m])
                for m in range(4):
                    emit_ctx(t, m, ops[m], aT_sb)
                attn8_cur = attn8_next

    nc.compile()
    return nc


_NC_CACHE = {}
TRACE = False
LAST_RESULT = {}


def kernel(feats, w_kq, b_kq, w_v, b_v, w_out, b_out, w_bot, b_bot):
    from concourse.bass_utils import run_bass_kernel_spmd

    feats = np.asarray(feats, dtype=np.float32)
    w_kq = np.asarray(w_kq, dtype=np.float32)
    b_kq = np.asarray(b_kq, dtype=np.float32)
    w_v = np.asarray(w_v, dtype=np.float32)
    b_v = np.asarray(b_v, dtype=np.float32)
    w_out = np.asarray(w_out, dtype=np.float32)
    b_out = np.asarray(b_out, dtype=np.float32)
    w_bot = np.asarray(w_bot, dtype=np.float32)
    b_bot = np.asarray(b_bot, dtype=np.float32)

    # host-side weight prep
    w_co = w_bot[:, :C] @ w_out                     # [CO, CK]
    b_co = w_bot[:, :C] @ b_out + b_bot + w_co @ b_v
    w_bf = w_bot[:, C:]                             # [CO, C]

    # wq8: [k, mc*512 + p*256 + i*128 + m] = 64*w_kq[mc*128+m, (2p+i)*128+k]
    arr = w_kq.T.reshape(2, 2, 128, 2, 128)         # [p, i, k, mc, m]
    wq8 = _f8(arr.transpose(2, 3, 0, 1, 4).reshape(128, 1024), S_W)
    # wv8 (moving operand for direct v^T): [k, p*512 + i*256 + o]
    arr = w_v.T.reshape(2, 2, 128, CK)              # [p, i, k, o]
    wv8 = _f8(arr.transpose(2, 0, 1, 3).reshape(128, 1024), S_W)

    # wbf8: [k, p*1024 + i*512 + o] = 2048*w_bf[o, (2p+i)*128+k]
    arr = w_bf.T.reshape(2, 2, 128, CO)             # [p, i, k, o]
    wbf_lay = arr.transpose(2, 0, 1, 3).reshape(128, 2048)
    wbf8 = _f8(wbf_lay, S_BF)
    wbfr8 = _f8(wbf_lay - wbf8.astype(np.float32) / np.float32(S_BF), S_BF)

    # wco8: [k, i*512 + o] = 64*w_co[o, i*128+k]
    arr = w_co.T.reshape(2, 128, CO)                # [i, k, o]
    wco8 = _f8(arr.transpose(1, 0, 2).reshape(128, 1024), S_WCO)

    if "nc" not in _NC_CACHE:
        _NC_CACHE["nc"] = _build_kernel()
    nc = _NC_CACHE["nc"]

    sc = np.float32(1.0 / np.sqrt(CK))
    in_maps = []
    for core in range(N_CORES):
        b, half = core // 2, core % 2
        h0 = half * H_SH
        pmat, inv_area = _build_pool_mat(h0)
        biases = np.zeros((128, 8), dtype=np.float32)
        biases[:, 0:2] = (S_W * b_kq).reshape(2, 128).T
        biases[:, 2:6] = (S_BF * b_co).reshape(4, 128).T
        biases[:S_TOT, 6] = inv_area * sc * (S_KEY / S_Q)
        biases[:S_TOT, 7] = inv_area * (S_VP / S_Q)
        shard = np.ascontiguousarray(
            feats[b, :, h0:h0 + H_SH, :]).reshape(C, NPIX)
        f8v = _f8(shard).astype(np.float32)
        fr8v = _f8(shard - f8v)
        f8v = f8v.astype(F8)
        # pair layouts [128, 2*NPIX]: chunk-pair planes side by side
        c4 = f8v.reshape(4, 128, NPIX)
        r4 = fr8v.reshape(4, 128, NPIX)
        f8cat = np.concatenate(
            [np.concatenate([c4[0], c4[1]], axis=1),
             np.concatenate([c4[2], c4[3]], axis=1),
             np.concatenate([r4[0], r4[1]], axis=1),
             np.concatenate([r4[2], r4[3]], axis=1)], axis=1)
        in_maps.append(dict(
            f8cat=np.ascontiguousarray(f8cat), wq8=wq8, wv8=wv8,
            wbf8=wbf8, wbfr8=wbfr8, wco8=wco8,
            pmat=pmat.reshape(W, H_SH * S_TOT), biases=biases,
        ))

    res = run_bass_kernel_spmd(
        nc, in_maps, list(range(N_CORES)), trace=TRACE
    )
    LAST_RESULT["res"] = res

    out = np.empty((B, CO, H, W), dtype=np.float32)
    for core in range(N_CORES):
        b, half = core // 2, core % 2
        h0 = half * H_SH
        out[b, :, h0:h0 + H_SH, :] = (
            np.asarray(res.results[core]["out"]).astype(np.float32)
            * OUT_UNSCALE
        ).reshape(CO, H_SH, W)
    return out
